# revision 26
# baseline (speedup 1.0000x reference)
"""AutoCorrelation block (FFT cross-correlation attention) on 8 Trainium2 cores.

Math (per batch b, faithfully reproducing the reference):
  qh = q @ Wq + bq, kh = k @ Wk + bk         (v projection is dead code)
  per channel c=(h,dh) (512 per batch):
    r = irfft(rfft(qh_c) * conj(rfft(kh_c)))   # circular cross-correlation
    top-8 lags d_k of r, softmax of the 8 values -> w_k
    agg_c[t] = sum_k w_k * qh_c[(t + d_k) % L]
  out = agg^T @ Wo + bo

Implementation: DFT-as-matmul with a stacked real cos/sin basis (the DFT matrix
is shared by all channels, so the whole FFT pipeline is dense PE work), DVE
max/max_index for top-8, and per-partition indirect-DMA gathers from a
time-doubled copy of qh for the mod-L rolls.

Sharding: data-parallel over batch. B == 8 == n_cores, one batch per core,
weights + DFT matrices replicated. No collectives.
"""

import numpy as np

import concourse.bass as bass
import concourse.bacc as bacc
import concourse.mybir as mybir
import concourse.tile as tile
from concourse.bass import IndirectOffsetOnAxis, ts
from concourse.bass_utils import run_bass_kernel_spmd

B, L, D = 8, 2048, 512
TOPK = 8
NF = 1025          # rfft bins for L=2048
FS = 2048          # stacked freq rows: 16 chunks of 128
IM0 = 1024         # sin(f) block at 1024+f (f=1..1023); slot 1024 = Nyquist cos
N_CORES = 8
KC = 4             # d_in chunks of 128
TM = 16            # time chunks of 128
CN = 4             # channel chunks of 128
FM = 16            # stacked-freq chunks of 128

F32 = mybir.dt.float32
F32R = mybir.dt.float32r
U32 = mybir.dt.uint32
BF16 = mybir.dt.bfloat16
F16 = mybir.dt.float16
AF = mybir.ActivationFunctionType
AX = mybir.AxisListType


def _build_dft_mats():
    # 16-chunk stacked real basis: cols 0..1023 = cos(2pi f t/L); col 1024 =
    # (-1)^t (Nyquist, reusing the identically-zero sin(0) slot); cols 1024+f =
    # sin(2pi f t/L) for f=1..1023. The frequency product treats chunk pairs
    # (j, 8+j) as (re, im); rows 0 and 1024 of Z get small post-fixes.
    t = np.arange(L)
    f = np.arange(1024)
    ang = (2.0 * np.pi / L) * ((t[:, None] * f[None, :]) % L)
    Cf = np.zeros((L, FS), np.float32)
    Cf[:, :1024] = np.cos(ang)
    Cf[:, 1024] = np.where(t % 2 == 0, 1.0, -1.0)
    Cf[:, 1025:] = np.sin(ang[:, 1:])
    # Mi is the UNSCALED inverse basis (entries in [-1, 1], exact in fp16);
    # the 2/L irfft scale is folded into the kf copy on-device (DC/Nyquist
    # rows get an extra 0.5 in the Z fix-up).
    ang2 = (2.0 * np.pi / L) * ((f[:, None] * t[None, :]) % L)
    Mi = np.zeros((FS, L), np.float32)
    Mi[0, :] = 1.0
    Mi[1:1024, :] = np.cos(ang2[1:])
    Mi[1024, :] = np.where(t % 2 == 0, 1.0, -1.0)
    Mi[1025:, :] = np.sin(ang2[1:])
    return Cf, Mi


def _kernel_body(tc, dr, out_ap, q2):
    nc = tc.nc

    w_pool = tc.alloc_tile_pool(name="weights", bufs=1)
    cf_pool = tc.alloc_tile_pool(name="cf", bufs=4, side="right")
    htd_pool = tc.alloc_tile_pool(name="htd", bufs=1, side="right")

    # ---- S1 inputs first so the PE can start ASAP ----
    qt_pool = tc.alloc_tile_pool(name="qt", bufs=1)
    qt = [qt_pool.tile([128, L], F16, tag=f"qt{i}", name=f"qt{i}") for i in range(KC)]
    kt = [qt_pool.tile([128, L], F16, tag=f"kt{i}", name=f"kt{i}") for i in range(KC)]

    # ---- constants (DMA order matters: the sync queue is in-order, so load
    # exactly what the first matmul group needs first) ----
    wqk_pool = tc.alloc_tile_pool(name="wqk", bufs=1)
    wq_t = wqk_pool.tile([128, KC * D], F16, tag="wqt", name="wqt")
    wk_t = wqk_pool.tile([128, KC * D], F16, tag="wkt", name="wkt")
    wo_t = w_pool.tile([128, KC * D], F16, tag="wot", name="wot")
    # quarter-tile interleaved loads: the first matmul needs only 384 KB
    for i in range(KC):
        nc.sync.dma_start(qt[i][:, 0:512], dr["qT"][ts(i, 128), 0:512])
        nc.sync.dma_start(wq_t[:, ts(i, D)], dr["Wq"][:, ts(i, D)])
    for q4 in range(1, 4):
        for i in range(KC):
            nc.sync.dma_start(qt[i][:, ts(q4, 512)], dr["qT"][ts(i, 128), ts(q4, 512)])
    # k-side + output weights on the Act HWDGE queue (arrive during S1-q)
    for i in range(KC):
        nc.scalar.dma_start(wk_t[:, ts(i, D)], dr["Wk"][:, ts(i, D)])
        nc.scalar.dma_start(kt[i][:, 0:512], dr["kT"][ts(i, 128), 0:512])
    for q4 in range(1, 4):
        for i in range(KC):
            nc.scalar.dma_start(kt[i][:, ts(q4, 512)], dr["kT"][ts(i, 128), ts(q4, 512)])
    nc.scalar.dma_start(wo_t[:, :], dr["Wo"][:, :])
    wq = [wq_t[:, ts(i, D)] for i in range(KC)]
    wk = [wk_t[:, ts(i, D)] for i in range(KC)]
    wo = [wo_t[:, ts(i, D)] for i in range(KC)]
    ones = w_pool.tile([1, 128], F16, tag="ones", name="ones")
    nc.scalar.dma_start(ones[:, :], dr["ones"][:, :])
    ident = w_pool.tile([128, 128], F16, tag="ident", name="ident")
    nc.scalar.dma_start(ident[:, :], dr["ident"][:, :])
    brow = {}
    brow["bkL2"] = w_pool.tile([1, D], F32R, tag="bkL2r", name="bkL2r")
    nc.scalar.dma_start(brow["bkL2"][:, :], dr["bkL2"][:, :])
    brow["bo"] = w_pool.tile([1, D], F16, tag="bor", name="bor")
    nc.scalar.dma_start(brow["bo"][:, :], dr["bo"][:, :])
    brow["bq16"] = w_pool.tile([1, D], F16, tag="bq16r", name="bq16r")
    nc.scalar.dma_start(brow["bq16"][:, :], dr["bq16"][:, :])

    htd_q = [htd_pool.tile([128, D], F16, tag=f"hq{m}", name=f"hq{m}") for m in range(TM)]
    htd_k = [htd_pool.tile([128, D], F16, tag=f"hk{m}", name=f"hk{m}") for m in range(TM)]

    # ---- S1/S2: projections (all-fp16 operands, fp32 PSUM accumulate) ----
    ps1 = tc.alloc_tile_pool(name="ps1", bufs=6, space="PSUM")
    qht_pool = tc.alloc_tile_pool(name="qht", bufs=2)

    # qh_td[t, c] = sum_di qT[di, t] * Wq[di, c] + bq (rank-1); bias inside
    # htd_q makes the DC-bin fix and the gather source both bias-correct.
    for grp in range(4):
        pss1 = [ps1.tile([128, D], F32, tag="p1", name="p1") for _ in range(4)]
        for kc in range(KC):
            for m4 in range(4):
                nc.tensor.matmul(
                    pss1[m4][:, :], qt[kc][:, ts(grp * 4 + m4, 128)], wq[kc],
                    start=(kc == 0), stop=False,
                )
        for m4 in range(4):
            nc.tensor.matmul(
                pss1[m4][:, :], ones[:, :], brow["bq16"][:, :],
                start=False, stop=True,
            )
            nc.scalar.activation(
                htd_q[grp * 4 + m4][:, :], pss1[m4][:, :], AF.Copy
            )
    # kh_td
    for m in range(TM):
        ps = ps1.tile([128, D], F32, tag="p1", name="p1")
        for kc in range(KC):
            nc.tensor.matmul(
                ps[:, :], kt[kc][:, ts(m, 128)], wk[kc],
                start=(kc == 0), stop=(kc == KC - 1),
            )
        nc.scalar.activation(htd_k[m][:, :], ps[:, :], AF.Copy)
    # qh_t[c, t] channel-major via PE transposes of the fp16 htd tiles
    # (1 cycle/row, 6x cheaper than re-projecting), DVE drains PSUM, then
    # doubled into q2 for the mod-L gathers.
    ps1t = tc.alloc_tile_pool(name="ps1t", bufs=2, space="PSUM")
    for mc in range(CN):
        qht = qht_pool.tile([128, L], F16, tag="qht", name="qht")
        for jg in range(4):
            pt = ps1t.tile([128, 512], F16, tag="pt", name="pt")
            for jj in range(4):
                m = 4 * jg + jj
                nc.tensor.transpose(
                    pt[:, ts(jj, 128)], htd_q[m][:, ts(mc, 128)], ident
                )
            nc.vector.tensor_copy(qht[:, ts(jg, 512)], pt[:, :])
        nc.gpsimd.dma_start(q2[ts(mc, 128), 0:L], qht[:, :])
        nc.gpsimd.dma_start(q2[ts(mc, 128), L : 2 * L], qht[:, :])

    ps1t.release()
    qht_pool.release()
    ps1.release()
    wqk_pool.release()
    qt_pool.release()

    # ---- S3+S4 fused: forward DFT with inline freq product ----
    # Qhat[fs, c] = sum_t Cf[t, fs] * qh_td[t, c]; pairs (j, 9+j) are produced
    # back-to-back so Z = Qhat * conj(Khat) is computed inline and the big
    # Qhat/Khat buffers never materialize.
    s_pool0 = tc.alloc_tile_pool(name="small0", bufs=1)
    iobs = []
    for mc in range(CN):
        iob = s_pool0.tile([128, 8], U32, tag=f"io{mc}", name=f"io{mc}")
        nc.gpsimd.iota(
            iob[:, :], pattern=[[0, 8]], base=mc * 128 * 2 * L,
            channel_multiplier=2 * L,
        )
        iobs.append(iob)
    # resident inverse basis: 4 fp16 n-chunks (8 MB), loaded on the Act HWDGE
    # queue while the forward DFT runs.
    mi_pool = tc.alloc_tile_pool(name="mi", bufs=1)
    mi_t = [mi_pool.tile([128, FM * 512], F16, tag=f"mi{n}", name=f"mi{n}")
            for n in range(4)]
    for n in range(4):
        nc.scalar.dma_start(mi_t[n][:, :], dr["Mi"][ts(n, 128), :])

    z_pool = tc.alloc_tile_pool(name="zfreq", bufs=1)
    f_pool = tc.alloc_tile_pool(name="fpair", bufs=4)
    ps3 = tc.alloc_tile_pool(name="ps3", bufs=2, space="PSUM")

    Z = [z_pool.tile([128, D], F16, tag=f"z{j}", name=f"z{j}") for j in range(FM)]

    def dft_chunk(m, scale):
        psq = ps3.tile([128, D], F32, tag="p3q", name="p3q")
        psk = ps3.tile([128, D], F32, tag="p3k", name="p3k")
        cf_t = cf_pool.tile([128, TM * 128], F16, tag="cf", name="cf")
        nc.sync.dma_start(cf_t[:, :], dr["Cf"][ts(m, 128), :])
        for kc in range(TM):
            nc.tensor.matmul(
                psq[:, :], cf_t[:, ts(kc, 128)], htd_q[kc][:, :],
                start=(kc == 0), stop=(kc == TM - 1),
            )
            nc.tensor.matmul(
                psk[:, :], cf_t[:, ts(kc, 128)], htd_k[kc][:, :],
                start=(kc == 0), stop=(kc == TM - 1),
            )
        qf = f_pool.tile([128, D], F32R, tag="qf", name="qf")
        kf = f_pool.tile([128, D], F32R, tag="kf", name="kf")
        nc.scalar.activation(qf[:, :], psq[:, :], AF.Copy)
        # fold the 2/L irfft scale into the k spectrum
        nc.scalar.activation(kf[:, :], psk[:, :], AF.Copy, scale=scale)
        return qf, kf

    for j in range(8):
        re, im = j, 8 + j
        qf_a, kf_a = dft_chunk(re, 2.0 / L)
        if j == 0:
            nc.vector.tensor_add(kf_a[0:1, :], kf_a[0:1, :], brow["bkL2"][:, :])
        qf_b, kf_b = dft_chunk(im, 2.0 / L)
        # Zre_j = Qre Kre + Qnim Knim ; Znim_j = Qnim Kre - Qre Knim
        t0 = f_pool.tile([128, D], F32R, tag="zt", name="zt")
        t1 = f_pool.tile([128, D], F32R, tag="zt", name="zt")
        nc.vector.tensor_mul(t0[:, :], qf_a[:, :], kf_a[:, :])
        nc.gpsimd.tensor_mul(t1[:, :], qf_b[:, :], kf_b[:, :])
        nc.vector.tensor_add(Z[re][:, :], t0[:, :], t1[:, :])
        t2 = f_pool.tile([128, D], F32R, tag="zt", name="zt")
        t3 = f_pool.tile([128, D], F32R, tag="zt", name="zt")
        nc.gpsimd.tensor_mul(t2[:, :], qf_b[:, :], kf_a[:, :])
        nc.vector.tensor_mul(t3[:, :], qf_a[:, :], kf_b[:, :])
        nc.vector.tensor_sub(Z[im][:, :], t2[:, :], t3[:, :])
        if j == 0:
            # row 0 of chunk 0 is the DC bin (no im partner): Z = Q0 * K0.
            # row 0 of chunk 8 is the Nyquist bin (real): Z = QN * KN.
            # Both are 1/L-scaled bins; kf carries 2/L, so halve.
            nc.vector.tensor_mul(t0[0:1, :], qf_a[0:1, :], kf_a[0:1, :])
            nc.vector.tensor_scalar_mul(Z[re][0:1, :], t0[0:1, :], 0.5)
            nc.vector.tensor_mul(t1[0:1, :], qf_b[0:1, :], kf_b[0:1, :])
            nc.vector.tensor_scalar_mul(Z[im][0:1, :], t1[0:1, :], 0.5)

    ps3.release()
    f_pool.release()
    htd_pool.release()
    cf_pool.release()

    # ---- S5/S6/S7 interleaved per channel chunk ----
    # inv-DFT(mc) on the PE; then its top-k + gather launches (DVE + SWDGE)
    # overlap inv-DFT(mc+1); wsum(mc) fills the PSUM-copy window of
    # inv-DFT(mc+2). Weights are folded into diag(w) fp16 stationaries.
    r_pool = tc.alloc_tile_pool(name="rcorr", bufs=1, side="right")
    psa = tc.alloc_tile_pool(name="psa", bufs=4, space="PSUM")
    ps5 = tc.alloc_tile_pool(name="ps5", bufs=4, space="PSUM")
    s_pool = tc.alloc_tile_pool(name="small", bufs=1)
    acc_pool = tc.alloc_tile_pool(name="acc", bufs=1, side="right")
    g_pool = tc.alloc_tile_pool(name="g", bufs=6)
    dg_pool = tc.alloc_tile_pool(name="dg", bufs=12)

    R = [r_pool.tile([128, L], F32, tag=f"r{m}", name=f"r{m}") for m in range(CN)]
    cand = [s_pool0.tile([128, 32], F32, tag=f"c{m}", name=f"c{m}") for m in range(CN)]
    acc = [acc_pool.tile([128, L], F16, tag=f"a{mc}", name=f"a{mc}") for mc in range(CN)]

    def inv_dft(mc):
        pss = [ps5.tile([128, 512], F32, tag="p5", name="p5") for _ in range(4)]
        for n in range(4):
            for kc in range(FM):
                nc.tensor.matmul(
                    pss[n][:, :], Z[kc][:, ts(mc, 128)], mi_t[n][:, ts(kc, 512)],
                    start=(kc == 0), stop=(kc == FM - 1),
                )
            nc.scalar.activation(R[mc][:, ts(n, 512)], pss[n][:, :], AF.Copy)
            nc.vector.max(out=cand[mc][:, ts(n, 8)], in_=R[mc][:, ts(n, 512)])

    def topk_gather(mc):
        vals = s_pool.tile([128, 8], F32, tag=f"v{mc}", name=f"v{mc}")
        nc.vector.max(out=vals[:, :], in_=cand[mc][:, :])
        idx = s_pool.tile([128, 8], U32, tag=f"i{mc}", name=f"i{mc}")
        nc.vector.max_index(out=idx[:, :], in_max=vals[:, :], in_values=R[mc][:, :])
        off = s_pool.tile([128, 8], U32, tag=f"o{mc}", name=f"o{mc}")
        nc.vector.tensor_add(off[:, :], idx[:, :], iobs[mc][:, :])
        gs = []
        for k in range(TOPK):
            g = g_pool.tile([128, L], F16, tag="g", name="g")
            gi = nc.gpsimd.indirect_dma_start(
                out=g[:, :],
                out_offset=None,
                in_=q2[:, :],
                in_offset=IndirectOffsetOnAxis(ap=off[:, k : k + 1], axis=1),
            )
            if k % 4:
                gi.ins.queue = f"qPoolDynamic{k % 4}"
            gs.append(g)
        negm = s_pool.tile([128, 1], F32, tag=f"nm{mc}", name=f"nm{mc}")
        nc.vector.tensor_scalar_mul(negm[:, :], vals[:, 0:1], -1.0)
        e = s_pool.tile([128, 8], F32, tag=f"e{mc}", name=f"e{mc}")
        nc.scalar.activation(e[:, :], vals[:, :], AF.Exp, bias=negm[:, :])
        ssum = s_pool.tile([128, 1], F32, tag=f"s{mc}", name=f"s{mc}")
        nc.vector.reduce_sum(out=ssum[:, :], in_=e[:, :], axis=AX.X)
        rs = s_pool.tile([128, 1], F32, tag=f"rs{mc}", name=f"rs{mc}")
        nc.vector.reciprocal(rs[:, :], ssum[:, :])
        wt = s_pool.tile([128, 8], F32, tag=f"w{mc}", name=f"w{mc}")
        nc.vector.tensor_scalar_mul(wt[:, :], e[:, :], rs[:, :])
        ds = []
        for k in range(TOPK):
            dg = dg_pool.tile([128, 128], F16, tag="dg", name="dg")
            nc.vector.tensor_scalar_mul(dg[:, :], ident[:, :], wt[:, k : k + 1])
            ds.append(dg)
        return gs, ds

    def wsum(mc, gs, ds):
        pacc = [psa.tile([128, 512], F32, tag="pa", name="pa") for _ in range(4)]
        for k in range(TOPK):
            for nsl in range(4):
                nc.tensor.matmul(
                    pacc[nsl][:, :], ds[k][:, :], gs[k][:, ts(nsl, 512)],
                    start=(k == 0), stop=(k == TOPK - 1),
                )
        for nsl in range(4):
            nc.scalar.activation(acc[mc][:, ts(nsl, 512)], pacc[nsl][:, :], AF.Copy)

    gd = {}
    inv_dft(0)
    gd[0] = topk_gather(0)
    inv_dft(1)
    gd[1] = topk_gather(1)
    wsum(0, *gd[0])
    inv_dft(2)
    gd[2] = topk_gather(2)
    wsum(1, *gd[1])
    inv_dft(3)
    gd[3] = topk_gather(3)
    wsum(2, *gd[2])

    ps5.release()
    po_pool = tc.alloc_tile_pool(name="po", bufs=1, space="PSUM")
    ot_pool = tc.alloc_tile_pool(name="ot", bufs=4, side="right")

    wsum(3, *gd[3])

    # ---- S8: output projection  out[t, :] = sum_c acc[c, t] * Wo[c, :] + bo.
    # Bias rank-1 first (independent of acc, fills the top-k latency gap),
    # then kc-outer per group of 4 m-chunks: matmuls on acc[kc] issue as each
    # acc completes, overlapping the remaining weighted-sum chains above.
    for grp in range(4):
        pss = [po_pool.tile([128, D], F32, tag=f"po{m4}", name=f"po{m4}")
               for m4 in range(4)]
        for m4 in range(4):
            nc.tensor.matmul(
                pss[m4][:, :], ones[:, :], brow["bo"][:, :], start=True, stop=False
            )
        for kc in range(CN):
            for m4 in range(4):
                nc.tensor.matmul(
                    pss[m4][:, :], acc[kc][:, ts(grp * 4 + m4, 128)], wo[kc],
                    start=False, stop=(kc == CN - 1),
                )
        for m4 in range(4):
            ot = ot_pool.tile([128, D], F32, tag="ot", name="ot")
            nc.scalar.activation(ot[:, :], pss[m4][:, :], AF.Copy)
            nc.sync.dma_start(out_ap[ts(grp * 4 + m4, 128), :], ot[:, :])

    ot_pool.release()
    po_pool.release()
    psa.release()
    dg_pool.release()
    g_pool.release()
    s_pool.release()
    z_pool.release()
    mi_pool.release()
    s_pool0.release()
    acc_pool.release()
    r_pool.release()
    w_pool.release()


def build_module():
    nc = bacc.Bacc(
        "TRN2",
        target_bir_lowering=False,
        debug=False,
        enable_asserts=False,
        num_devices=N_CORES,
        num_swdge_queues=4,
    )
    dr = {}

    def din(name, shape, dt=F32R):
        dr[name] = nc.dram_tensor(name, shape, dt, kind="ExternalInput").ap()

    din("qT", [D, L], F16)
    din("kT", [D, L], F16)
    din("Wq", [128, KC * D], F16)   # tiled: [p, kc*D+j] = W[kc*128+p, j]
    din("Wk", [128, KC * D], F16)
    din("Wo", [128, KC * D], F16)
    din("bo", [1, D], F16)
    din("bq16", [1, D], F16)
    din("bkL2", [1, D])
    din("ones", [1, 128], F16)
    din("ident", [128, 128], F16)
    din("Cf", [FM * 128, TM * 128], F16)   # [m*128+p, kc*128+j] = Cf[kc*128+p, m*128+j]
    din("Mi", [4 * 128, FM * 512], F16)    # [n*128+p, kc*512+j] = Mi[kc*128+p, n*512+j]
    out_ap = nc.dram_tensor("out", [L, D], F32, kind="ExternalOutput").ap()
    q2 = nc.dram_tensor("q2", [D, 2 * L], F16, kind="Internal").ap()

    with tile.TileContext(nc, trace_sim=False) as tc:
        _kernel_body(tc, dr, out_ap, q2)
    nc.compile()
    return nc


_NC_CACHE = {}


def _tile_w(W):
    return np.ascontiguousarray(
        np.asarray(W, np.float32).reshape(KC, 128, D).transpose(1, 0, 2).reshape(128, KC * D)
    )


def make_in_maps(q, k, Wq, bq, Wk, bk, Wo, bo):
    Cf, Mi = _build_dft_mats()
    # pre-tile so each stage does one big contiguous DMA per chunk column
    Cf = np.ascontiguousarray(
        Cf.reshape(TM, 128, FM, 128).transpose(2, 1, 0, 3).reshape(FM * 128, TM * 128)
    )
    Mi = np.ascontiguousarray(
        Mi.reshape(FM, 128, 4, 512).transpose(2, 1, 0, 3).reshape(4 * 128, FM * 512)
    ).astype(np.float16)
    f32 = np.float32
    shared = {
        "Wq": _tile_w(Wq).astype(np.float16),
        "Wk": _tile_w(Wk).astype(np.float16),
        "Wo": _tile_w(Wo).astype(np.float16),
        "bo": np.ascontiguousarray(bo, f32).reshape(1, D).astype(np.float16),
        "bq16": np.ascontiguousarray(bq, f32).reshape(1, D).astype(np.float16),
        "bkL2": np.ascontiguousarray(np.asarray(bk, f32) * 2.0, f32).reshape(1, D),
        "ones": np.ones((1, 128), np.float16),
        "ident": np.eye(128, dtype=np.float16),
        "Cf": Cf.astype(np.float16),
        "Mi": Mi,
    }
    in_maps = []
    for b in range(B):
        m = dict(shared)
        m["qT"] = np.ascontiguousarray(np.asarray(q[b], f32).T).astype(np.float16)
        m["kT"] = np.ascontiguousarray(np.asarray(k[b], f32).T).astype(np.float16)
        in_maps.append(m)
    return in_maps


def kernel(q, k, v, Wq, bq, Wk, bk, Wv, bv, Wo, bo, _want_results=False,
           _trace=False, **_ignored):
    if "nc" not in _NC_CACHE:
        _NC_CACHE["nc"] = build_module()
    nc = _NC_CACHE["nc"]
    in_maps = make_in_maps(q, k, Wq, bq, Wk, bk, Wo, bo)
    res = run_bass_kernel_spmd(
        nc, in_maps, core_ids=list(range(N_CORES)), trace=_trace
    )
    out = np.stack([np.asarray(res.results[b]["out"], np.float32) for b in range(B)])
    if _want_results:
        return out, res
    return out


if __name__ == "__main__":
    # smoke test with random data
    rng = np.random.default_rng(0)
    q = rng.standard_normal((B, L, D), np.float32)
    k = rng.standard_normal((B, L, D), np.float32)
    s = 1.0 / np.sqrt(D)
    Wq = rng.standard_normal((D, D), np.float32) * s
    Wk = rng.standard_normal((D, D), np.float32) * s
    Wo = rng.standard_normal((D, D), np.float32) * s
    z = np.zeros(D, np.float32)
    out = kernel(q, k, None, Wq, z, Wk, z, None, None, Wo, z)
    print("out", out.shape, out.dtype, float(np.abs(out).sum()))



# revision 29
# speedup vs baseline: 90862.7040x; 90862.7040x over previous
"""AutoCorrelation block (FFT cross-correlation attention) on 8 Trainium2 cores.

Math (per batch b, faithfully reproducing the reference):
  qh = q @ Wq + bq, kh = k @ Wk + bk         (v projection is dead code)
  per channel c=(h,dh) (512 per batch):
    r = irfft(rfft(qh_c) * conj(rfft(kh_c)))   # circular cross-correlation
    top-8 lags d_k of r, softmax of the 8 values -> w_k
    agg_c[t] = sum_k w_k * qh_c[(t + d_k) % L]
  out = agg^T @ Wo + bo

Implementation: DFT-as-matmul with a stacked real cos/sin basis (the DFT matrix
is shared by all channels, so the whole FFT pipeline is dense PE work), DVE
max/max_index for top-8, and per-partition indirect-DMA gathers from a
time-doubled copy of qh for the mod-L rolls.

Sharding: data-parallel over batch. B == 8 == n_cores, one batch per core,
weights + DFT matrices replicated. No collectives.
"""

import numpy as np

import concourse.bass as bass
import concourse.bacc as bacc
import concourse.mybir as mybir
import concourse.tile as tile
from concourse.bass import IndirectOffsetOnAxis, ts
from concourse.bass_utils import run_bass_kernel_spmd

B, L, D = 8, 2048, 512
TOPK = 8
NF = 1025          # rfft bins for L=2048
FS = 2048          # stacked freq rows: 16 chunks of 128
IM0 = 1024         # sin(f) block at 1024+f (f=1..1023); slot 1024 = Nyquist cos
N_CORES = 8
KC = 4             # d_in chunks of 128
TM = 16            # time chunks of 128
CN = 4             # channel chunks of 128
FM = 16            # stacked-freq chunks of 128

F32 = mybir.dt.float32
F32R = mybir.dt.float32r
U32 = mybir.dt.uint32
BF16 = mybir.dt.bfloat16
F16 = mybir.dt.float16
AF = mybir.ActivationFunctionType
AX = mybir.AxisListType


def _build_dft_mats():
    # 16-chunk stacked real basis: cols 0..1023 = cos(2pi f t/L); col 1024 =
    # (-1)^t (Nyquist, reusing the identically-zero sin(0) slot); cols 1024+f =
    # sin(2pi f t/L) for f=1..1023. The frequency product treats chunk pairs
    # (j, 8+j) as (re, im); rows 0 and 1024 of Z get small post-fixes.
    t = np.arange(L)
    f = np.arange(1024)
    ang = (2.0 * np.pi / L) * ((t[:, None] * f[None, :]) % L)
    Cf = np.zeros((L, FS), np.float32)
    Cf[:, :1024] = np.cos(ang)
    Cf[:, 1024] = np.where(t % 2 == 0, 1.0, -1.0)
    Cf[:, 1025:] = np.sin(ang[:, 1:])
    # Mi is the UNSCALED inverse basis (entries in [-1, 1], exact in fp16);
    # the 2/L irfft scale is folded into the kf copy on-device (DC/Nyquist
    # rows get an extra 0.5 in the Z fix-up).
    ang2 = (2.0 * np.pi / L) * ((f[:, None] * t[None, :]) % L)
    Mi = np.zeros((FS, L), np.float32)
    Mi[0, :] = 1.0
    Mi[1:1024, :] = np.cos(ang2[1:])
    Mi[1024, :] = np.where(t % 2 == 0, 1.0, -1.0)
    Mi[1025:, :] = np.sin(ang2[1:])
    return Cf, Mi


def _kernel_body(tc, dr, out_ap, q2):
    nc = tc.nc

    w_pool = tc.alloc_tile_pool(name="weights", bufs=1)
    cf_pool = tc.alloc_tile_pool(name="cf", bufs=4, side="right")
    htd_pool = tc.alloc_tile_pool(name="htd", bufs=1, side="right")

    # ---- S1 inputs first so the PE can start ASAP ----
    qt_pool = tc.alloc_tile_pool(name="qt", bufs=1)
    qt = [qt_pool.tile([128, L], F16, tag=f"qt{i}", name=f"qt{i}") for i in range(KC)]
    kt = [qt_pool.tile([128, L], F16, tag=f"kt{i}", name=f"kt{i}") for i in range(KC)]

    # ---- constants (DMA order matters: the sync queue is in-order, so load
    # exactly what the first matmul group needs first) ----
    wqk_pool = tc.alloc_tile_pool(name="wqk", bufs=1)
    wq_t = wqk_pool.tile([128, KC * D], F16, tag="wqt", name="wqt")
    wk_t = wqk_pool.tile([128, KC * D], F16, tag="wkt", name="wkt")
    wo_t = w_pool.tile([128, KC * D], F16, tag="wot", name="wot")
    # tiny constants first (37 KB): bias rows and ident must not sit behind
    # megabyte loads — the grp-0 bias matmul needs them at ~14us.
    ones = w_pool.tile([1, 128], F16, tag="ones", name="ones")
    nc.sync.dma_start(ones[:, :], dr["ones"][:, :])
    ident = w_pool.tile([128, 128], F16, tag="ident", name="ident")
    nc.sync.dma_start(ident[:, :], dr["ident"][:, :])
    brow = {}
    brow["bq16"] = w_pool.tile([1, D], F16, tag="bq16r", name="bq16r")
    nc.sync.dma_start(brow["bq16"][:, :], dr["bq16"][:, :])
    brow["bkL2"] = w_pool.tile([1, D], F32R, tag="bkL2r", name="bkL2r")
    nc.sync.dma_start(brow["bkL2"][:, :], dr["bkL2"][:, :])
    brow["bo"] = w_pool.tile([1, D], F16, tag="bor", name="bor")
    nc.sync.dma_start(brow["bo"][:, :], dr["bo"][:, :])
    # quarter-tile interleaved loads: the first matmul needs only 384 KB
    for i in range(KC):
        nc.sync.dma_start(qt[i][:, 0:512], dr["qT"][ts(i, 128), 0:512])
        nc.sync.dma_start(wq_t[:, ts(i, D)], dr["Wq"][:, ts(i, D)])
    for q4 in range(1, 4):
        for i in range(KC):
            nc.sync.dma_start(qt[i][:, ts(q4, 512)], dr["qT"][ts(i, 128), ts(q4, 512)])
    # k-side + output weights on the Act HWDGE queue (arrive during S1-q)
    for i in range(KC):
        nc.scalar.dma_start(wk_t[:, ts(i, D)], dr["Wk"][:, ts(i, D)])
        nc.scalar.dma_start(kt[i][:, 0:512], dr["kT"][ts(i, 128), 0:512])
    for q4 in range(1, 4):
        for i in range(KC):
            nc.scalar.dma_start(kt[i][:, ts(q4, 512)], dr["kT"][ts(i, 128), ts(q4, 512)])
    nc.scalar.dma_start(wo_t[:, :], dr["Wo"][:, :])
    wq = [wq_t[:, ts(i, D)] for i in range(KC)]
    wk = [wk_t[:, ts(i, D)] for i in range(KC)]
    wo = [wo_t[:, ts(i, D)] for i in range(KC)]

    htd_q = [htd_pool.tile([128, D], F16, tag=f"hq{m}", name=f"hq{m}") for m in range(TM)]
    htd_k = [htd_pool.tile([128, D], F16, tag=f"hk{m}", name=f"hk{m}") for m in range(TM)]

    # ---- S1/S2: projections (all-fp16 operands, fp32 PSUM accumulate) ----
    ps1 = tc.alloc_tile_pool(name="ps1", bufs=6, space="PSUM")
    qht_pool = tc.alloc_tile_pool(name="qht", bufs=2)

    # qh_td[t, c] = sum_di qT[di, t] * Wq[di, c] + bq (rank-1); bias inside
    # htd_q makes the DC-bin fix and the gather source both bias-correct.
    for grp in range(4):
        pss1 = [ps1.tile([128, D], F32, tag="p1", name="p1") for _ in range(4)]
        for kc in range(KC):
            for m4 in range(4):
                nc.tensor.matmul(
                    pss1[m4][:, :], qt[kc][:, ts(grp * 4 + m4, 128)], wq[kc],
                    start=(kc == 0), stop=False,
                )
        for m4 in range(4):
            nc.tensor.matmul(
                pss1[m4][:, :], ones[:, :], brow["bq16"][:, :],
                start=False, stop=True,
            )
            nc.scalar.activation(
                htd_q[grp * 4 + m4][:, :], pss1[m4][:, :], AF.Copy
            )
    # kh_td
    for m in range(TM):
        ps = ps1.tile([128, D], F32, tag="p1", name="p1")
        for kc in range(KC):
            nc.tensor.matmul(
                ps[:, :], kt[kc][:, ts(m, 128)], wk[kc],
                start=(kc == 0), stop=(kc == KC - 1),
            )
        nc.scalar.activation(htd_k[m][:, :], ps[:, :], AF.Copy)
    # qh_t[c, t] channel-major via PE transposes of the fp16 htd tiles
    # (1 cycle/row, 6x cheaper than re-projecting), DVE drains PSUM, then
    # doubled into q2 for the mod-L gathers.
    ps1t = tc.alloc_tile_pool(name="ps1t", bufs=2, space="PSUM")
    for mc in range(CN):
        qht = qht_pool.tile([128, L], F16, tag="qht", name="qht")
        for jg in range(4):
            pt = ps1t.tile([128, 512], F16, tag="pt", name="pt")
            for jj in range(4):
                m = 4 * jg + jj
                nc.tensor.transpose(
                    pt[:, ts(jj, 128)], htd_q[m][:, ts(mc, 128)], ident
                )
            nc.vector.tensor_copy(qht[:, ts(jg, 512)], pt[:, :])
        nc.gpsimd.dma_start(q2[ts(mc, 128), 0:L], qht[:, :])
        nc.gpsimd.dma_start(q2[ts(mc, 128), L : 2 * L], qht[:, :])

    ps1t.release()
    qht_pool.release()
    ps1.release()
    wqk_pool.release()
    qt_pool.release()

    # ---- S3+S4 fused: forward DFT with inline freq product ----
    # Qhat[fs, c] = sum_t Cf[t, fs] * qh_td[t, c]; pairs (j, 9+j) are produced
    # back-to-back so Z = Qhat * conj(Khat) is computed inline and the big
    # Qhat/Khat buffers never materialize.
    s_pool0 = tc.alloc_tile_pool(name="small0", bufs=1)
    iobs = []
    for mc in range(CN):
        iob = s_pool0.tile([128, 8], U32, tag=f"io{mc}", name=f"io{mc}")
        nc.gpsimd.iota(
            iob[:, :], pattern=[[0, 8]], base=mc * 128 * 2 * L,
            channel_multiplier=2 * L,
        )
        iobs.append(iob)
    # resident inverse basis: 4 fp16 n-chunks (8 MB), loaded on the Act HWDGE
    # queue while the forward DFT runs.
    mi_pool = tc.alloc_tile_pool(name="mi", bufs=1)
    mi_t = [mi_pool.tile([128, FM * 512], F16, tag=f"mi{n}", name=f"mi{n}")
            for n in range(4)]
    for n in range(4):
        nc.scalar.dma_start(mi_t[n][:, :], dr["Mi"][ts(n, 128), :])

    z_pool = tc.alloc_tile_pool(name="zfreq", bufs=1)
    f_pool = tc.alloc_tile_pool(name="fpair", bufs=4)
    ps3 = tc.alloc_tile_pool(name="ps3", bufs=2, space="PSUM")

    Z = [z_pool.tile([128, D], F16, tag=f"z{j}", name=f"z{j}") for j in range(FM)]

    def dft_chunk(m, scale):
        psq = ps3.tile([128, D], F32, tag="p3q", name="p3q")
        psk = ps3.tile([128, D], F32, tag="p3k", name="p3k")
        cf_t = cf_pool.tile([128, TM * 128], F16, tag="cf", name="cf")
        nc.sync.dma_start(cf_t[:, :], dr["Cf"][ts(m, 128), :])
        for kc in range(TM):
            nc.tensor.matmul(
                psq[:, :], cf_t[:, ts(kc, 128)], htd_q[kc][:, :],
                start=(kc == 0), stop=(kc == TM - 1),
            )
            nc.tensor.matmul(
                psk[:, :], cf_t[:, ts(kc, 128)], htd_k[kc][:, :],
                start=(kc == 0), stop=(kc == TM - 1),
            )
        qf = f_pool.tile([128, D], F32R, tag="qf", name="qf")
        kf = f_pool.tile([128, D], F32R, tag="kf", name="kf")
        nc.scalar.activation(qf[:, :], psq[:, :], AF.Copy)
        # fold the 2/L irfft scale into the k spectrum
        nc.scalar.activation(kf[:, :], psk[:, :], AF.Copy, scale=scale)
        return qf, kf

    for j in range(8):
        re, im = j, 8 + j
        qf_a, kf_a = dft_chunk(re, 2.0 / L)
        if j == 0:
            nc.vector.tensor_add(kf_a[0:1, :], kf_a[0:1, :], brow["bkL2"][:, :])
        qf_b, kf_b = dft_chunk(im, 2.0 / L)
        # Zre_j = Qre Kre + Qnim Knim ; Znim_j = Qnim Kre - Qre Knim
        t0 = f_pool.tile([128, D], F32R, tag="zt", name="zt")
        t1 = f_pool.tile([128, D], F32R, tag="zt", name="zt")
        nc.vector.tensor_mul(t0[:, :], qf_a[:, :], kf_a[:, :])
        nc.gpsimd.tensor_mul(t1[:, :], qf_b[:, :], kf_b[:, :])
        nc.vector.tensor_add(Z[re][:, :], t0[:, :], t1[:, :])
        t2 = f_pool.tile([128, D], F32R, tag="zt", name="zt")
        t3 = f_pool.tile([128, D], F32R, tag="zt", name="zt")
        nc.gpsimd.tensor_mul(t2[:, :], qf_b[:, :], kf_a[:, :])
        nc.vector.tensor_mul(t3[:, :], qf_a[:, :], kf_b[:, :])
        nc.vector.tensor_sub(Z[im][:, :], t2[:, :], t3[:, :])
        if j == 0:
            # row 0 of chunk 0 is the DC bin (no im partner): Z = Q0 * K0.
            # row 0 of chunk 8 is the Nyquist bin (real): Z = QN * KN.
            # Both are 1/L-scaled bins; kf carries 2/L, so halve.
            nc.vector.tensor_mul(t0[0:1, :], qf_a[0:1, :], kf_a[0:1, :])
            nc.vector.tensor_scalar_mul(Z[re][0:1, :], t0[0:1, :], 0.5)
            nc.vector.tensor_mul(t1[0:1, :], qf_b[0:1, :], kf_b[0:1, :])
            nc.vector.tensor_scalar_mul(Z[im][0:1, :], t1[0:1, :], 0.5)

    ps3.release()
    f_pool.release()
    htd_pool.release()
    cf_pool.release()

    # ---- S5/S6/S7 interleaved per channel chunk ----
    # inv-DFT(mc) on the PE; then its top-k + gather launches (DVE + SWDGE)
    # overlap inv-DFT(mc+1); wsum(mc) fills the PSUM-copy window of
    # inv-DFT(mc+2). Weights are folded into diag(w) fp16 stationaries.
    r_pool = tc.alloc_tile_pool(name="rcorr", bufs=1, side="right")
    psa = tc.alloc_tile_pool(name="psa", bufs=4, space="PSUM")
    ps5 = tc.alloc_tile_pool(name="ps5", bufs=4, space="PSUM")
    s_pool = tc.alloc_tile_pool(name="small", bufs=1)
    acc_pool = tc.alloc_tile_pool(name="acc", bufs=1, side="right")
    g_pool = tc.alloc_tile_pool(name="g", bufs=6)
    dg_pool = tc.alloc_tile_pool(name="dg", bufs=12)

    R = [r_pool.tile([128, L], F32, tag=f"r{m}", name=f"r{m}") for m in range(CN)]
    cand = [s_pool0.tile([128, 32], F32, tag=f"c{m}", name=f"c{m}") for m in range(CN)]
    acc = [acc_pool.tile([128, L], F16, tag=f"a{mc}", name=f"a{mc}") for mc in range(CN)]

    def inv_dft(mc):
        pss = [ps5.tile([128, 512], F32, tag="p5", name="p5") for _ in range(4)]
        for n in range(4):
            for kc in range(FM):
                nc.tensor.matmul(
                    pss[n][:, :], Z[kc][:, ts(mc, 128)], mi_t[n][:, ts(kc, 512)],
                    start=(kc == 0), stop=(kc == FM - 1),
                )
            nc.scalar.activation(R[mc][:, ts(n, 512)], pss[n][:, :], AF.Copy)
            nc.vector.max(out=cand[mc][:, ts(n, 8)], in_=R[mc][:, ts(n, 512)])

    def topk_gather(mc):
        vals = s_pool.tile([128, 8], F32, tag=f"v{mc}", name=f"v{mc}")
        nc.vector.max(out=vals[:, :], in_=cand[mc][:, :])
        idx = s_pool.tile([128, 8], U32, tag=f"i{mc}", name=f"i{mc}")
        nc.vector.max_index(out=idx[:, :], in_max=vals[:, :], in_values=R[mc][:, :])
        off = s_pool.tile([128, 8], U32, tag=f"o{mc}", name=f"o{mc}")
        nc.vector.tensor_add(off[:, :], idx[:, :], iobs[mc][:, :])
        gs = []
        for k in range(TOPK):
            g = g_pool.tile([128, L], F16, tag="g", name="g")
            gi = nc.gpsimd.indirect_dma_start(
                out=g[:, :],
                out_offset=None,
                in_=q2[:, :],
                in_offset=IndirectOffsetOnAxis(ap=off[:, k : k + 1], axis=1),
            )
            if k % 4:
                gi.ins.queue = f"qPoolDynamic{k % 4}"
            gs.append(g)
        negm = s_pool.tile([128, 1], F32, tag=f"nm{mc}", name=f"nm{mc}")
        nc.vector.tensor_scalar_mul(negm[:, :], vals[:, 0:1], -1.0)
        e = s_pool.tile([128, 8], F32, tag=f"e{mc}", name=f"e{mc}")
        nc.scalar.activation(e[:, :], vals[:, :], AF.Exp, bias=negm[:, :])
        ssum = s_pool.tile([128, 1], F32, tag=f"s{mc}", name=f"s{mc}")
        nc.vector.reduce_sum(out=ssum[:, :], in_=e[:, :], axis=AX.X)
        rs = s_pool.tile([128, 1], F32, tag=f"rs{mc}", name=f"rs{mc}")
        nc.vector.reciprocal(rs[:, :], ssum[:, :])
        wt = s_pool.tile([128, 8], F32, tag=f"w{mc}", name=f"w{mc}")
        nc.vector.tensor_scalar_mul(wt[:, :], e[:, :], rs[:, :])
        ds = []
        for k in range(TOPK):
            dg = dg_pool.tile([128, 128], F16, tag="dg", name="dg")
            nc.vector.tensor_scalar_mul(dg[:, :], ident[:, :], wt[:, k : k + 1])
            ds.append(dg)
        return gs, ds

    def wsum(mc, gs, ds):
        pacc = [psa.tile([128, 512], F32, tag="pa", name="pa") for _ in range(4)]
        for k in range(TOPK):
            for nsl in range(4):
                nc.tensor.matmul(
                    pacc[nsl][:, :], ds[k][:, :], gs[k][:, ts(nsl, 512)],
                    start=(k == 0), stop=(k == TOPK - 1),
                )
        for nsl in range(4):
            nc.scalar.activation(acc[mc][:, ts(nsl, 512)], pacc[nsl][:, :], AF.Copy)

    gd = {}
    inv_dft(0)
    gd[0] = topk_gather(0)
    inv_dft(1)
    gd[1] = topk_gather(1)
    wsum(0, *gd[0])
    inv_dft(2)
    gd[2] = topk_gather(2)
    wsum(1, *gd[1])
    inv_dft(3)
    gd[3] = topk_gather(3)
    wsum(2, *gd[2])

    ps5.release()
    po_pool = tc.alloc_tile_pool(name="po", bufs=1, space="PSUM")
    ot_pool = tc.alloc_tile_pool(name="ot", bufs=4, side="right")

    wsum(3, *gd[3])

    # ---- S8: output projection  out[t, :] = sum_c acc[c, t] * Wo[c, :] + bo.
    # Bias rank-1 first (independent of acc, fills the top-k latency gap),
    # then kc-outer per group of 4 m-chunks: matmuls on acc[kc] issue as each
    # acc completes, overlapping the remaining weighted-sum chains above.
    for grp in range(4):
        pss = [po_pool.tile([128, D], F32, tag=f"po{m4}", name=f"po{m4}")
               for m4 in range(4)]
        for m4 in range(4):
            nc.tensor.matmul(
                pss[m4][:, :], ones[:, :], brow["bo"][:, :], start=True, stop=False
            )
        for kc in range(CN):
            for m4 in range(4):
                nc.tensor.matmul(
                    pss[m4][:, :], acc[kc][:, ts(grp * 4 + m4, 128)], wo[kc],
                    start=False, stop=(kc == CN - 1),
                )
        for m4 in range(4):
            ot = ot_pool.tile([128, D], F16, tag="ot", name="ot")
            nc.scalar.activation(ot[:, :], pss[m4][:, :], AF.Copy)
            nc.sync.dma_start(out_ap[ts(grp * 4 + m4, 128), :], ot[:, :])

    ot_pool.release()
    po_pool.release()
    psa.release()
    dg_pool.release()
    g_pool.release()
    s_pool.release()
    z_pool.release()
    mi_pool.release()
    s_pool0.release()
    acc_pool.release()
    r_pool.release()
    w_pool.release()


def build_module():
    nc = bacc.Bacc(
        "TRN2",
        target_bir_lowering=False,
        debug=False,
        enable_asserts=False,
        num_devices=N_CORES,
        num_swdge_queues=4,
    )
    dr = {}

    def din(name, shape, dt=F32R):
        dr[name] = nc.dram_tensor(name, shape, dt, kind="ExternalInput").ap()

    din("qT", [D, L], F16)
    din("kT", [D, L], F16)
    din("Wq", [128, KC * D], F16)   # tiled: [p, kc*D+j] = W[kc*128+p, j]
    din("Wk", [128, KC * D], F16)
    din("Wo", [128, KC * D], F16)
    din("bo", [1, D], F16)
    din("bq16", [1, D], F16)
    din("bkL2", [1, D])
    din("ones", [1, 128], F16)
    din("ident", [128, 128], F16)
    din("Cf", [FM * 128, TM * 128], F16)   # [m*128+p, kc*128+j] = Cf[kc*128+p, m*128+j]
    din("Mi", [4 * 128, FM * 512], F16)    # [n*128+p, kc*512+j] = Mi[kc*128+p, n*512+j]
    out_ap = nc.dram_tensor("out", [L, D], F16, kind="ExternalOutput").ap()
    q2 = nc.dram_tensor("q2", [D, 2 * L], F16, kind="Internal").ap()

    with tile.TileContext(nc, trace_sim=False) as tc:
        _kernel_body(tc, dr, out_ap, q2)
    nc.compile()
    return nc


_NC_CACHE = {}


def _tile_w(W):
    return np.ascontiguousarray(
        np.asarray(W, np.float32).reshape(KC, 128, D).transpose(1, 0, 2).reshape(128, KC * D)
    )


def make_in_maps(q, k, Wq, bq, Wk, bk, Wo, bo):
    Cf, Mi = _build_dft_mats()
    # pre-tile so each stage does one big contiguous DMA per chunk column
    Cf = np.ascontiguousarray(
        Cf.reshape(TM, 128, FM, 128).transpose(2, 1, 0, 3).reshape(FM * 128, TM * 128)
    )
    Mi = np.ascontiguousarray(
        Mi.reshape(FM, 128, 4, 512).transpose(2, 1, 0, 3).reshape(4 * 128, FM * 512)
    ).astype(np.float16)
    f32 = np.float32
    shared = {
        "Wq": _tile_w(Wq).astype(np.float16),
        "Wk": _tile_w(Wk).astype(np.float16),
        "Wo": _tile_w(Wo).astype(np.float16),
        "bo": np.ascontiguousarray(bo, f32).reshape(1, D).astype(np.float16),
        "bq16": np.ascontiguousarray(bq, f32).reshape(1, D).astype(np.float16),
        "bkL2": np.ascontiguousarray(np.asarray(bk, f32) * 2.0, f32).reshape(1, D),
        "ones": np.ones((1, 128), np.float16),
        "ident": np.eye(128, dtype=np.float16),
        "Cf": Cf.astype(np.float16),
        "Mi": Mi,
    }
    in_maps = []
    for b in range(B):
        m = dict(shared)
        m["qT"] = np.ascontiguousarray(np.asarray(q[b], f32).T).astype(np.float16)
        m["kT"] = np.ascontiguousarray(np.asarray(k[b], f32).T).astype(np.float16)
        in_maps.append(m)
    return in_maps


def kernel(q, k, v, Wq, bq, Wk, bk, Wv, bv, Wo, bo, _want_results=False,
           _trace=False, **_ignored):
    if "nc" not in _NC_CACHE:
        _NC_CACHE["nc"] = build_module()
    nc = _NC_CACHE["nc"]
    in_maps = make_in_maps(q, k, Wq, bq, Wk, bk, Wo, bo)
    res = run_bass_kernel_spmd(
        nc, in_maps, core_ids=list(range(N_CORES)), trace=_trace
    )
    out = np.stack([np.asarray(res.results[b]["out"], np.float32) for b in range(B)])
    if _want_results:
        return out, res
    return out


if __name__ == "__main__":
    # smoke test with random data
    rng = np.random.default_rng(0)
    q = rng.standard_normal((B, L, D), np.float32)
    k = rng.standard_normal((B, L, D), np.float32)
    s = 1.0 / np.sqrt(D)
    Wq = rng.standard_normal((D, D), np.float32) * s
    Wk = rng.standard_normal((D, D), np.float32) * s
    Wo = rng.standard_normal((D, D), np.float32) * s
    z = np.zeros(D, np.float32)
    out = kernel(q, k, None, Wq, z, Wk, z, None, None, Wo, z)
    print("out", out.shape, out.dtype, float(np.abs(out).sum()))



# revision 30
# speedup vs baseline: 92390.4949x; 1.0168x over previous
"""AutoCorrelation block (FFT cross-correlation attention) on 8 Trainium2 cores.

Math (per batch b, faithfully reproducing the reference):
  qh = q @ Wq + bq, kh = k @ Wk + bk         (v projection is dead code)
  per channel c=(h,dh) (512 per batch):
    r = irfft(rfft(qh_c) * conj(rfft(kh_c)))   # circular cross-correlation
    top-8 lags d_k of r, softmax of the 8 values -> w_k
    agg_c[t] = sum_k w_k * qh_c[(t + d_k) % L]
  out = agg^T @ Wo + bo

Implementation: DFT-as-matmul with a stacked real cos/sin basis (the DFT matrix
is shared by all channels, so the whole FFT pipeline is dense PE work), DVE
max/max_index for top-8, and per-partition indirect-DMA gathers from a
time-doubled copy of qh for the mod-L rolls.

Sharding: data-parallel over batch. B == 8 == n_cores, one batch per core,
weights + DFT matrices replicated. No collectives.
"""

import numpy as np

import concourse.bass as bass
import concourse.bacc as bacc
import concourse.mybir as mybir
import concourse.tile as tile
from concourse.bass import IndirectOffsetOnAxis, ts
from concourse.bass_utils import run_bass_kernel_spmd

B, L, D = 8, 2048, 512
TOPK = 8
NF = 1025          # rfft bins for L=2048
FS = 2048          # stacked freq rows: 16 chunks of 128
IM0 = 1024         # sin(f) block at 1024+f (f=1..1023); slot 1024 = Nyquist cos
N_CORES = 8
KC = 4             # d_in chunks of 128
TM = 16            # time chunks of 128
CN = 4             # channel chunks of 128
FM = 16            # stacked-freq chunks of 128

F32 = mybir.dt.float32
F32R = mybir.dt.float32r
U32 = mybir.dt.uint32
BF16 = mybir.dt.bfloat16
F16 = mybir.dt.float16
AF = mybir.ActivationFunctionType
AX = mybir.AxisListType


def _build_dft_mats():
    # 16-chunk stacked real basis: cols 0..1023 = cos(2pi f t/L); col 1024 =
    # (-1)^t (Nyquist, reusing the identically-zero sin(0) slot); cols 1024+f =
    # sin(2pi f t/L) for f=1..1023. The frequency product treats chunk pairs
    # (j, 8+j) as (re, im); rows 0 and 1024 of Z get small post-fixes.
    t = np.arange(L)
    f = np.arange(1024)
    ang = (2.0 * np.pi / L) * ((t[:, None] * f[None, :]) % L)
    Cf = np.zeros((L, FS), np.float32)
    Cf[:, :1024] = np.cos(ang)
    Cf[:, 1024] = np.where(t % 2 == 0, 1.0, -1.0)
    Cf[:, 1025:] = np.sin(ang[:, 1:])
    # Mi is the UNSCALED inverse basis (entries in [-1, 1], exact in fp16);
    # the 2/L irfft scale is folded into the kf copy on-device (DC/Nyquist
    # rows get an extra 0.5 in the Z fix-up).
    ang2 = (2.0 * np.pi / L) * ((f[:, None] * t[None, :]) % L)
    Mi = np.zeros((FS, L), np.float32)
    Mi[0, :] = 1.0
    Mi[1:1024, :] = np.cos(ang2[1:])
    Mi[1024, :] = np.where(t % 2 == 0, 1.0, -1.0)
    Mi[1025:, :] = np.sin(ang2[1:])
    return Cf, Mi


def _kernel_body(tc, dr, out_ap, q2):
    nc = tc.nc

    w_pool = tc.alloc_tile_pool(name="weights", bufs=1)
    cf_pool = tc.alloc_tile_pool(name="cf", bufs=4, side="right")
    htd_pool = tc.alloc_tile_pool(name="htd", bufs=1, side="right")

    # ---- S1 inputs first so the PE can start ASAP ----
    qt_pool = tc.alloc_tile_pool(name="qt", bufs=1)
    qt = [qt_pool.tile([128, L], F16, tag=f"qt{i}", name=f"qt{i}") for i in range(KC)]
    kt = [qt_pool.tile([128, L], F16, tag=f"kt{i}", name=f"kt{i}") for i in range(KC)]

    # ---- constants (DMA order matters: the sync queue is in-order, so load
    # exactly what the first matmul group needs first) ----
    wqk_pool = tc.alloc_tile_pool(name="wqk", bufs=1)
    wq_t = wqk_pool.tile([128, KC * D], F16, tag="wqt", name="wqt")
    wk_t = wqk_pool.tile([128, KC * D], F16, tag="wkt", name="wkt")
    wo_t = w_pool.tile([128, KC * D], F16, tag="wot", name="wot")
    # tiny constants first (37 KB): bias rows and ident must not sit behind
    # megabyte loads — the grp-0 bias matmul needs them at ~14us.
    ones = w_pool.tile([1, 128], F16, tag="ones", name="ones")
    nc.sync.dma_start(ones[:, :], dr["ones"][:, :])
    ident = w_pool.tile([128, 128], F16, tag="ident", name="ident")
    nc.sync.dma_start(ident[:, :], dr["ident"][:, :])
    brow = {}
    brow["bq16"] = w_pool.tile([1, D], F16, tag="bq16r", name="bq16r")
    nc.sync.dma_start(brow["bq16"][:, :], dr["bq16"][:, :])
    brow["bkL2"] = w_pool.tile([1, D], F32R, tag="bkL2r", name="bkL2r")
    nc.sync.dma_start(brow["bkL2"][:, :], dr["bkL2"][:, :])
    brow["bo"] = w_pool.tile([1, D], F16, tag="bor", name="bor")
    nc.sync.dma_start(brow["bo"][:, :], dr["bo"][:, :])
    # quarter-tile interleaved loads: the first matmul needs only 384 KB
    for i in range(KC):
        nc.sync.dma_start(qt[i][:, 0:512], dr["qT"][ts(i, 128), 0:512])
        nc.sync.dma_start(wq_t[:, ts(i, D)], dr["Wq"][:, ts(i, D)])
    for q4 in range(1, 4):
        for i in range(KC):
            nc.sync.dma_start(qt[i][:, ts(q4, 512)], dr["qT"][ts(i, 128), ts(q4, 512)])
    # k-side after q-side on the same queue: full bandwidth for the critical
    # path, and kt still lands well before the kh projection (~31us)
    for i in range(KC):
        nc.sync.dma_start(wk_t[:, ts(i, D)], dr["Wk"][:, ts(i, D)])
        nc.sync.dma_start(kt[i][:, 0:512], dr["kT"][ts(i, 128), 0:512])
    for q4 in range(1, 4):
        for i in range(KC):
            nc.sync.dma_start(kt[i][:, ts(q4, 512)], dr["kT"][ts(i, 128), ts(q4, 512)])
    nc.scalar.dma_start(wo_t[:, :], dr["Wo"][:, :])
    wq = [wq_t[:, ts(i, D)] for i in range(KC)]
    wk = [wk_t[:, ts(i, D)] for i in range(KC)]
    wo = [wo_t[:, ts(i, D)] for i in range(KC)]

    htd_q = [htd_pool.tile([128, D], F16, tag=f"hq{m}", name=f"hq{m}") for m in range(TM)]
    htd_k = [htd_pool.tile([128, D], F16, tag=f"hk{m}", name=f"hk{m}") for m in range(TM)]

    # ---- S1/S2: projections (all-fp16 operands, fp32 PSUM accumulate) ----
    ps1 = tc.alloc_tile_pool(name="ps1", bufs=6, space="PSUM")
    qht_pool = tc.alloc_tile_pool(name="qht", bufs=2)

    # qh_td[t, c] = sum_di qT[di, t] * Wq[di, c] + bq (rank-1); bias inside
    # htd_q makes the DC-bin fix and the gather source both bias-correct.
    for grp in range(4):
        pss1 = [ps1.tile([128, D], F32, tag="p1", name="p1") for _ in range(4)]
        for kc in range(KC):
            for m4 in range(4):
                nc.tensor.matmul(
                    pss1[m4][:, :], qt[kc][:, ts(grp * 4 + m4, 128)], wq[kc],
                    start=(kc == 0), stop=False,
                )
        for m4 in range(4):
            nc.tensor.matmul(
                pss1[m4][:, :], ones[:, :], brow["bq16"][:, :],
                start=False, stop=True,
            )
            nc.scalar.activation(
                htd_q[grp * 4 + m4][:, :], pss1[m4][:, :], AF.Copy
            )
    # kh_td
    for m in range(TM):
        ps = ps1.tile([128, D], F32, tag="p1", name="p1")
        for kc in range(KC):
            nc.tensor.matmul(
                ps[:, :], kt[kc][:, ts(m, 128)], wk[kc],
                start=(kc == 0), stop=(kc == KC - 1),
            )
        nc.scalar.activation(htd_k[m][:, :], ps[:, :], AF.Copy)
    # qh_t[c, t] channel-major via PE transposes of the fp16 htd tiles
    # (1 cycle/row, 6x cheaper than re-projecting), DVE drains PSUM, then
    # doubled into q2 for the mod-L gathers.
    ps1t = tc.alloc_tile_pool(name="ps1t", bufs=2, space="PSUM")
    for mc in range(CN):
        qht = qht_pool.tile([128, L], F16, tag="qht", name="qht")
        for jg in range(4):
            pt = ps1t.tile([128, 512], F16, tag="pt", name="pt")
            for jj in range(4):
                m = 4 * jg + jj
                nc.tensor.transpose(
                    pt[:, ts(jj, 128)], htd_q[m][:, ts(mc, 128)], ident
                )
            nc.vector.tensor_copy(qht[:, ts(jg, 512)], pt[:, :])
        nc.gpsimd.dma_start(q2[ts(mc, 128), 0:L], qht[:, :])
        nc.gpsimd.dma_start(q2[ts(mc, 128), L : 2 * L], qht[:, :])

    ps1t.release()
    qht_pool.release()
    ps1.release()
    wqk_pool.release()
    qt_pool.release()

    # ---- S3+S4 fused: forward DFT with inline freq product ----
    # Qhat[fs, c] = sum_t Cf[t, fs] * qh_td[t, c]; pairs (j, 9+j) are produced
    # back-to-back so Z = Qhat * conj(Khat) is computed inline and the big
    # Qhat/Khat buffers never materialize.
    s_pool0 = tc.alloc_tile_pool(name="small0", bufs=1)
    iobs = []
    for mc in range(CN):
        iob = s_pool0.tile([128, 8], U32, tag=f"io{mc}", name=f"io{mc}")
        nc.gpsimd.iota(
            iob[:, :], pattern=[[0, 8]], base=mc * 128 * 2 * L,
            channel_multiplier=2 * L,
        )
        iobs.append(iob)
    # resident inverse basis: 4 fp16 n-chunks (8 MB), loaded on the Act HWDGE
    # queue while the forward DFT runs.
    mi_pool = tc.alloc_tile_pool(name="mi", bufs=1)
    mi_t = [mi_pool.tile([128, FM * 512], F16, tag=f"mi{n}", name=f"mi{n}")
            for n in range(4)]
    for n in range(4):
        nc.scalar.dma_start(mi_t[n][:, :], dr["Mi"][ts(n, 128), :])

    z_pool = tc.alloc_tile_pool(name="zfreq", bufs=1)
    f_pool = tc.alloc_tile_pool(name="fpair", bufs=4)
    ps3 = tc.alloc_tile_pool(name="ps3", bufs=2, space="PSUM")

    Z = [z_pool.tile([128, D], F16, tag=f"z{j}", name=f"z{j}") for j in range(FM)]

    def dft_chunk(m, scale):
        psq = ps3.tile([128, D], F32, tag="p3q", name="p3q")
        psk = ps3.tile([128, D], F32, tag="p3k", name="p3k")
        cf_t = cf_pool.tile([128, TM * 128], F16, tag="cf", name="cf")
        nc.sync.dma_start(cf_t[:, :], dr["Cf"][ts(m, 128), :])
        for kc in range(TM):
            nc.tensor.matmul(
                psq[:, :], cf_t[:, ts(kc, 128)], htd_q[kc][:, :],
                start=(kc == 0), stop=(kc == TM - 1),
            )
            nc.tensor.matmul(
                psk[:, :], cf_t[:, ts(kc, 128)], htd_k[kc][:, :],
                start=(kc == 0), stop=(kc == TM - 1),
            )
        qf = f_pool.tile([128, D], F32R, tag="qf", name="qf")
        kf = f_pool.tile([128, D], F32R, tag="kf", name="kf")
        nc.scalar.activation(qf[:, :], psq[:, :], AF.Copy)
        # fold the 2/L irfft scale into the k spectrum
        nc.scalar.activation(kf[:, :], psk[:, :], AF.Copy, scale=scale)
        return qf, kf

    for j in range(8):
        re, im = j, 8 + j
        qf_a, kf_a = dft_chunk(re, 2.0 / L)
        if j == 0:
            nc.vector.tensor_add(kf_a[0:1, :], kf_a[0:1, :], brow["bkL2"][:, :])
        qf_b, kf_b = dft_chunk(im, 2.0 / L)
        # Zre_j = Qre Kre + Qnim Knim ; Znim_j = Qnim Kre - Qre Knim
        t0 = f_pool.tile([128, D], F32R, tag="zt", name="zt")
        t1 = f_pool.tile([128, D], F32R, tag="zt", name="zt")
        nc.vector.tensor_mul(t0[:, :], qf_a[:, :], kf_a[:, :])
        nc.gpsimd.tensor_mul(t1[:, :], qf_b[:, :], kf_b[:, :])
        nc.vector.tensor_add(Z[re][:, :], t0[:, :], t1[:, :])
        t2 = f_pool.tile([128, D], F32R, tag="zt", name="zt")
        t3 = f_pool.tile([128, D], F32R, tag="zt", name="zt")
        nc.gpsimd.tensor_mul(t2[:, :], qf_b[:, :], kf_a[:, :])
        nc.vector.tensor_mul(t3[:, :], qf_a[:, :], kf_b[:, :])
        nc.vector.tensor_sub(Z[im][:, :], t2[:, :], t3[:, :])
        if j == 0:
            # row 0 of chunk 0 is the DC bin (no im partner): Z = Q0 * K0.
            # row 0 of chunk 8 is the Nyquist bin (real): Z = QN * KN.
            # Both are 1/L-scaled bins; kf carries 2/L, so halve.
            nc.vector.tensor_mul(t0[0:1, :], qf_a[0:1, :], kf_a[0:1, :])
            nc.vector.tensor_scalar_mul(Z[re][0:1, :], t0[0:1, :], 0.5)
            nc.vector.tensor_mul(t1[0:1, :], qf_b[0:1, :], kf_b[0:1, :])
            nc.vector.tensor_scalar_mul(Z[im][0:1, :], t1[0:1, :], 0.5)

    ps3.release()
    f_pool.release()
    htd_pool.release()
    cf_pool.release()

    # ---- S5/S6/S7 interleaved per channel chunk ----
    # inv-DFT(mc) on the PE; then its top-k + gather launches (DVE + SWDGE)
    # overlap inv-DFT(mc+1); wsum(mc) fills the PSUM-copy window of
    # inv-DFT(mc+2). Weights are folded into diag(w) fp16 stationaries.
    r_pool = tc.alloc_tile_pool(name="rcorr", bufs=1, side="right")
    psa = tc.alloc_tile_pool(name="psa", bufs=4, space="PSUM")
    ps5 = tc.alloc_tile_pool(name="ps5", bufs=4, space="PSUM")
    s_pool = tc.alloc_tile_pool(name="small", bufs=1)
    acc_pool = tc.alloc_tile_pool(name="acc", bufs=1, side="right")
    g_pool = tc.alloc_tile_pool(name="g", bufs=6)
    dg_pool = tc.alloc_tile_pool(name="dg", bufs=12)

    R = [r_pool.tile([128, L], F32, tag=f"r{m}", name=f"r{m}") for m in range(CN)]
    cand = [s_pool0.tile([128, 32], F32, tag=f"c{m}", name=f"c{m}") for m in range(CN)]
    acc = [acc_pool.tile([128, L], F16, tag=f"a{mc}", name=f"a{mc}") for mc in range(CN)]

    # accumulate in Z-production order (re/im pairs) so the last fwd
    # products are needed last
    KORD = [j for p in range(8) for j in (p, 8 + p)]

    def inv_dft(mc):
        pss = [ps5.tile([128, 512], F32, tag="p5", name="p5") for _ in range(4)]
        for n in range(4):
            for i, kc in enumerate(KORD):
                nc.tensor.matmul(
                    pss[n][:, :], Z[kc][:, ts(mc, 128)], mi_t[n][:, ts(kc, 512)],
                    start=(i == 0), stop=(i == FM - 1),
                )
            nc.scalar.activation(R[mc][:, ts(n, 512)], pss[n][:, :], AF.Copy)
            nc.vector.max(out=cand[mc][:, ts(n, 8)], in_=R[mc][:, ts(n, 512)])

    def topk_gather(mc):
        vals = s_pool.tile([128, 8], F32, tag=f"v{mc}", name=f"v{mc}")
        nc.vector.max(out=vals[:, :], in_=cand[mc][:, :])
        idx = s_pool.tile([128, 8], U32, tag=f"i{mc}", name=f"i{mc}")
        nc.vector.max_index(out=idx[:, :], in_max=vals[:, :], in_values=R[mc][:, :])
        off = s_pool.tile([128, 8], U32, tag=f"o{mc}", name=f"o{mc}")
        nc.vector.tensor_add(off[:, :], idx[:, :], iobs[mc][:, :])
        gs = []
        for k in range(TOPK):
            g = g_pool.tile([128, L], F16, tag="g", name="g")
            gi = nc.gpsimd.indirect_dma_start(
                out=g[:, :],
                out_offset=None,
                in_=q2[:, :],
                in_offset=IndirectOffsetOnAxis(ap=off[:, k : k + 1], axis=1),
            )
            if k % 4:
                gi.ins.queue = f"qPoolDynamic{k % 4}"
            gs.append(g)
        negm = s_pool.tile([128, 1], F32, tag=f"nm{mc}", name=f"nm{mc}")
        nc.vector.tensor_scalar_mul(negm[:, :], vals[:, 0:1], -1.0)
        e = s_pool.tile([128, 8], F32, tag=f"e{mc}", name=f"e{mc}")
        nc.scalar.activation(e[:, :], vals[:, :], AF.Exp, bias=negm[:, :])
        ssum = s_pool.tile([128, 1], F32, tag=f"s{mc}", name=f"s{mc}")
        nc.vector.reduce_sum(out=ssum[:, :], in_=e[:, :], axis=AX.X)
        rs = s_pool.tile([128, 1], F32, tag=f"rs{mc}", name=f"rs{mc}")
        nc.vector.reciprocal(rs[:, :], ssum[:, :])
        wt = s_pool.tile([128, 8], F32, tag=f"w{mc}", name=f"w{mc}")
        nc.vector.tensor_scalar_mul(wt[:, :], e[:, :], rs[:, :])
        ds = []
        for k in range(TOPK):
            dg = dg_pool.tile([128, 128], F16, tag="dg", name="dg")
            nc.vector.tensor_scalar_mul(dg[:, :], ident[:, :], wt[:, k : k + 1])
            ds.append(dg)
        return gs, ds

    def wsum(mc, gs, ds):
        pacc = [psa.tile([128, 512], F32, tag="pa", name="pa") for _ in range(4)]
        for k in range(TOPK):
            for nsl in range(4):
                nc.tensor.matmul(
                    pacc[nsl][:, :], ds[k][:, :], gs[k][:, ts(nsl, 512)],
                    start=(k == 0), stop=(k == TOPK - 1),
                )
        for nsl in range(4):
            nc.scalar.activation(acc[mc][:, ts(nsl, 512)], pacc[nsl][:, :], AF.Copy)

    gd = {}
    inv_dft(0)
    gd[0] = topk_gather(0)
    inv_dft(1)
    gd[1] = topk_gather(1)
    wsum(0, *gd[0])
    inv_dft(2)
    gd[2] = topk_gather(2)
    wsum(1, *gd[1])
    inv_dft(3)
    gd[3] = topk_gather(3)
    wsum(2, *gd[2])

    ps5.release()
    po_pool = tc.alloc_tile_pool(name="po", bufs=1, space="PSUM")
    ot_pool = tc.alloc_tile_pool(name="ot", bufs=4, side="right")

    wsum(3, *gd[3])

    # ---- S8: output projection  out[t, :] = sum_c acc[c, t] * Wo[c, :] + bo.
    # Bias rank-1 first (independent of acc, fills the top-k latency gap),
    # then kc-outer per group of 4 m-chunks: matmuls on acc[kc] issue as each
    # acc completes, overlapping the remaining weighted-sum chains above.
    for grp in range(4):
        pss = [po_pool.tile([128, D], F32, tag=f"po{m4}", name=f"po{m4}")
               for m4 in range(4)]
        for m4 in range(4):
            nc.tensor.matmul(
                pss[m4][:, :], ones[:, :], brow["bo"][:, :], start=True, stop=False
            )
        for kc in range(CN):
            for m4 in range(4):
                nc.tensor.matmul(
                    pss[m4][:, :], acc[kc][:, ts(grp * 4 + m4, 128)], wo[kc],
                    start=False, stop=(kc == CN - 1),
                )
        for m4 in range(4):
            ot = ot_pool.tile([128, D], F16, tag="ot", name="ot")
            nc.scalar.activation(ot[:, :], pss[m4][:, :], AF.Copy)
            nc.sync.dma_start(out_ap[ts(grp * 4 + m4, 128), :], ot[:, :])

    ot_pool.release()
    po_pool.release()
    psa.release()
    dg_pool.release()
    g_pool.release()
    s_pool.release()
    z_pool.release()
    mi_pool.release()
    s_pool0.release()
    acc_pool.release()
    r_pool.release()
    w_pool.release()


def build_module():
    nc = bacc.Bacc(
        "TRN2",
        target_bir_lowering=False,
        debug=False,
        enable_asserts=False,
        num_devices=N_CORES,
        num_swdge_queues=4,
    )
    dr = {}

    def din(name, shape, dt=F32R):
        dr[name] = nc.dram_tensor(name, shape, dt, kind="ExternalInput").ap()

    din("qT", [D, L], F16)
    din("kT", [D, L], F16)
    din("Wq", [128, KC * D], F16)   # tiled: [p, kc*D+j] = W[kc*128+p, j]
    din("Wk", [128, KC * D], F16)
    din("Wo", [128, KC * D], F16)
    din("bo", [1, D], F16)
    din("bq16", [1, D], F16)
    din("bkL2", [1, D])
    din("ones", [1, 128], F16)
    din("ident", [128, 128], F16)
    din("Cf", [FM * 128, TM * 128], F16)   # [m*128+p, kc*128+j] = Cf[kc*128+p, m*128+j]
    din("Mi", [4 * 128, FM * 512], F16)    # [n*128+p, kc*512+j] = Mi[kc*128+p, n*512+j]
    out_ap = nc.dram_tensor("out", [L, D], F16, kind="ExternalOutput").ap()
    q2 = nc.dram_tensor("q2", [D, 2 * L], F16, kind="Internal").ap()

    with tile.TileContext(nc, trace_sim=False) as tc:
        _kernel_body(tc, dr, out_ap, q2)
    nc.compile()
    return nc


_NC_CACHE = {}


def _tile_w(W):
    return np.ascontiguousarray(
        np.asarray(W, np.float32).reshape(KC, 128, D).transpose(1, 0, 2).reshape(128, KC * D)
    )


def make_in_maps(q, k, Wq, bq, Wk, bk, Wo, bo):
    Cf, Mi = _build_dft_mats()
    # pre-tile so each stage does one big contiguous DMA per chunk column
    Cf = np.ascontiguousarray(
        Cf.reshape(TM, 128, FM, 128).transpose(2, 1, 0, 3).reshape(FM * 128, TM * 128)
    )
    Mi = np.ascontiguousarray(
        Mi.reshape(FM, 128, 4, 512).transpose(2, 1, 0, 3).reshape(4 * 128, FM * 512)
    ).astype(np.float16)
    f32 = np.float32
    shared = {
        "Wq": _tile_w(Wq).astype(np.float16),
        "Wk": _tile_w(Wk).astype(np.float16),
        "Wo": _tile_w(Wo).astype(np.float16),
        "bo": np.ascontiguousarray(bo, f32).reshape(1, D).astype(np.float16),
        "bq16": np.ascontiguousarray(bq, f32).reshape(1, D).astype(np.float16),
        "bkL2": np.ascontiguousarray(np.asarray(bk, f32) * 2.0, f32).reshape(1, D),
        "ones": np.ones((1, 128), np.float16),
        "ident": np.eye(128, dtype=np.float16),
        "Cf": Cf.astype(np.float16),
        "Mi": Mi,
    }
    in_maps = []
    for b in range(B):
        m = dict(shared)
        m["qT"] = np.ascontiguousarray(np.asarray(q[b], f32).T).astype(np.float16)
        m["kT"] = np.ascontiguousarray(np.asarray(k[b], f32).T).astype(np.float16)
        in_maps.append(m)
    return in_maps


def kernel(q, k, v, Wq, bq, Wk, bk, Wv, bv, Wo, bo, _want_results=False,
           _trace=False, **_ignored):
    if "nc" not in _NC_CACHE:
        _NC_CACHE["nc"] = build_module()
    nc = _NC_CACHE["nc"]
    in_maps = make_in_maps(q, k, Wq, bq, Wk, bk, Wo, bo)
    res = run_bass_kernel_spmd(
        nc, in_maps, core_ids=list(range(N_CORES)), trace=_trace
    )
    out = np.stack([np.asarray(res.results[b]["out"], np.float32) for b in range(B)])
    if _want_results:
        return out, res
    return out


if __name__ == "__main__":
    # smoke test with random data
    rng = np.random.default_rng(0)
    q = rng.standard_normal((B, L, D), np.float32)
    k = rng.standard_normal((B, L, D), np.float32)
    s = 1.0 / np.sqrt(D)
    Wq = rng.standard_normal((D, D), np.float32) * s
    Wk = rng.standard_normal((D, D), np.float32) * s
    Wo = rng.standard_normal((D, D), np.float32) * s
    z = np.zeros(D, np.float32)
    out = kernel(q, k, None, Wq, z, Wk, z, None, None, Wo, z)
    print("out", out.shape, out.dtype, float(np.abs(out).sum()))



# revision 31
# speedup vs baseline: 94714.1469x; 1.0252x over previous
"""AutoCorrelation block (FFT cross-correlation attention) on 8 Trainium2 cores.

Math (per batch b, faithfully reproducing the reference):
  qh = q @ Wq + bq, kh = k @ Wk + bk         (v projection is dead code)
  per channel c=(h,dh) (512 per batch):
    r = irfft(rfft(qh_c) * conj(rfft(kh_c)))   # circular cross-correlation
    top-8 lags d_k of r, softmax of the 8 values -> w_k
    agg_c[t] = sum_k w_k * qh_c[(t + d_k) % L]
  out = agg^T @ Wo + bo

Implementation: DFT-as-matmul with a stacked real cos/sin basis (the DFT matrix
is shared by all channels, so the whole FFT pipeline is dense PE work), DVE
max/max_index for top-8, and per-partition indirect-DMA gathers from a
time-doubled copy of qh for the mod-L rolls.

Sharding: data-parallel over batch. B == 8 == n_cores, one batch per core,
weights + DFT matrices replicated. No collectives.
"""

import numpy as np

import concourse.bass as bass
import concourse.bacc as bacc
import concourse.mybir as mybir
import concourse.tile as tile
from concourse.bass import IndirectOffsetOnAxis, ts
from concourse.bass_utils import run_bass_kernel_spmd

B, L, D = 8, 2048, 512
TOPK = 8
NF = 1025          # rfft bins for L=2048
FS = 2048          # stacked freq rows: 16 chunks of 128
IM0 = 1024         # sin(f) block at 1024+f (f=1..1023); slot 1024 = Nyquist cos
N_CORES = 8
KC = 4             # d_in chunks of 128
TM = 16            # time chunks of 128
CN = 4             # channel chunks of 128
FM = 16            # stacked-freq chunks of 128

F32 = mybir.dt.float32
F32R = mybir.dt.float32r
U32 = mybir.dt.uint32
BF16 = mybir.dt.bfloat16
F16 = mybir.dt.float16
AF = mybir.ActivationFunctionType
AX = mybir.AxisListType


def _build_dft_mats():
    # 16-chunk stacked real basis: cols 0..1023 = cos(2pi f t/L); col 1024 =
    # (-1)^t (Nyquist, reusing the identically-zero sin(0) slot); cols 1024+f =
    # sin(2pi f t/L) for f=1..1023. The frequency product treats chunk pairs
    # (j, 8+j) as (re, im); rows 0 and 1024 of Z get small post-fixes.
    t = np.arange(L)
    f = np.arange(1024)
    ang = (2.0 * np.pi / L) * ((t[:, None] * f[None, :]) % L)
    Cf = np.zeros((L, FS), np.float32)
    Cf[:, :1024] = np.cos(ang)
    Cf[:, 1024] = np.where(t % 2 == 0, 1.0, -1.0)
    Cf[:, 1025:] = np.sin(ang[:, 1:])
    # Mi is the UNSCALED inverse basis (entries in [-1, 1], exact in fp16);
    # the 2/L irfft scale is folded into the kf copy on-device (DC/Nyquist
    # rows get an extra 0.5 in the Z fix-up).
    ang2 = (2.0 * np.pi / L) * ((f[:, None] * t[None, :]) % L)
    Mi = np.zeros((FS, L), np.float32)
    Mi[0, :] = 1.0
    Mi[1:1024, :] = np.cos(ang2[1:])
    Mi[1024, :] = np.where(t % 2 == 0, 1.0, -1.0)
    Mi[1025:, :] = np.sin(ang2[1:])
    return Cf, Mi


def _kernel_body(tc, dr, out_ap, q2):
    nc = tc.nc

    w_pool = tc.alloc_tile_pool(name="weights", bufs=1)
    cf_pool = tc.alloc_tile_pool(name="cf", bufs=4, side="right")
    htd_pool = tc.alloc_tile_pool(name="htd", bufs=1, side="right")

    # ---- S1 inputs first so the PE can start ASAP ----
    qt_pool = tc.alloc_tile_pool(name="qt", bufs=1)
    qt = [qt_pool.tile([128, L], F16, tag=f"qt{i}", name=f"qt{i}") for i in range(KC)]
    kt = [qt_pool.tile([128, L], F16, tag=f"kt{i}", name=f"kt{i}") for i in range(KC)]

    # ---- constants (DMA order matters: the sync queue is in-order, so load
    # exactly what the first matmul group needs first) ----
    wqk_pool = tc.alloc_tile_pool(name="wqk", bufs=1)
    wq_t = wqk_pool.tile([128, KC * D], F16, tag="wqt", name="wqt")
    wk_t = wqk_pool.tile([128, KC * D], F16, tag="wkt", name="wkt")
    wo_t = w_pool.tile([128, KC * D], F16, tag="wot", name="wot")
    # tiny constants first (37 KB): bias rows and ident must not sit behind
    # megabyte loads — the grp-0 bias matmul needs them at ~14us.
    ident = w_pool.tile([128, 128], F16, tag="ident", name="ident")
    nc.sync.dma_start(ident[:, :], dr["ident"][:, :])
    brow = {}
    for nm in ("bqL", "bkL2"):
        brow[nm] = w_pool.tile([1, D], F32R, tag=f"{nm}r", name=f"{nm}r")
        nc.sync.dma_start(brow[nm][:, :], dr[nm][:, :])
    bqcol = w_pool.tile([128, CN], F32, tag="bqc", name="bqc")
    for c in range(CN):
        nc.sync.dma_start(bqcol[:, c : c + 1], dr["bqc"][ts(c, 128), :])
    bocol = w_pool.tile([128, CN], F32, tag="boc", name="boc")
    for c in range(CN):
        nc.sync.dma_start(bocol[:, c : c + 1], dr["boc"][ts(c, 128), :])
    # quarter-tile interleaved loads split across both HW queues: the first
    # matmul group needs only ~0.8 MB, ~0.4 MB per queue
    for i in range(KC):
        eng = nc.sync if i % 2 == 0 else nc.scalar
        eng.dma_start(qt[i][:, 0:512], dr["qT"][ts(i, 128), 0:512])
        eng.dma_start(wq_t[:, ts(i, D)], dr["Wq"][:, ts(i, D)])
    for q4 in range(1, 4):
        for i in range(KC):
            nc.sync.dma_start(qt[i][:, ts(q4, 512)], dr["qT"][ts(i, 128), ts(q4, 512)])
    # k-side after q-side on the same queue: full bandwidth for the critical
    # path, and kt still lands well before the kh projection (~31us)
    for i in range(KC):
        nc.sync.dma_start(wk_t[:, ts(i, D)], dr["Wk"][:, ts(i, D)])
        nc.sync.dma_start(kt[i][:, 0:512], dr["kT"][ts(i, 128), 0:512])
    for q4 in range(1, 4):
        for i in range(KC):
            nc.sync.dma_start(kt[i][:, ts(q4, 512)], dr["kT"][ts(i, 128), ts(q4, 512)])
    nc.scalar.dma_start(wo_t[:, :], dr["Wo"][:, :])
    wq = [wq_t[:, ts(i, D)] for i in range(KC)]
    wk = [wk_t[:, ts(i, D)] for i in range(KC)]
    wo = [wo_t[:, ts(i, D)] for i in range(KC)]

    htd_q = [htd_pool.tile([128, D], F16, tag=f"hq{m}", name=f"hq{m}") for m in range(TM)]
    htd_k = [htd_pool.tile([128, D], F16, tag=f"hk{m}", name=f"hk{m}") for m in range(TM)]

    # ---- S1/S2: projections (all-fp16 operands, fp32 PSUM accumulate) ----
    ps1 = tc.alloc_tile_pool(name="ps1", bufs=6, space="PSUM")
    qht_pool = tc.alloc_tile_pool(name="qht", bufs=2)

    # qh_td[t, c] = sum_di qT[di, t] * Wq[di, c]; bias is applied in the
    # channel-major transpose copies (per-partition there) and via the DC-bin
    # fix in the forward DFT.
    for grp in range(4):
        pss1 = [ps1.tile([128, D], F32, tag="p1", name="p1") for _ in range(4)]
        for kc in range(KC):
            for m4 in range(4):
                nc.tensor.matmul(
                    pss1[m4][:, :], qt[kc][:, ts(grp * 4 + m4, 128)], wq[kc],
                    start=(kc == 0), stop=(kc == KC - 1),
                )
        for m4 in range(4):
            nc.scalar.activation(
                htd_q[grp * 4 + m4][:, :], pss1[m4][:, :], AF.Copy
            )
    # kh_td
    for m in range(TM):
        ps = ps1.tile([128, D], F32, tag="p1", name="p1")
        for kc in range(KC):
            nc.tensor.matmul(
                ps[:, :], kt[kc][:, ts(m, 128)], wk[kc],
                start=(kc == 0), stop=(kc == KC - 1),
            )
        nc.scalar.activation(htd_k[m][:, :], ps[:, :], AF.Copy)
    # qh_t[c, t] channel-major via PE transposes of the fp16 htd tiles
    # (1 cycle/row, 6x cheaper than re-projecting), DVE drains PSUM, then
    # doubled into q2 for the mod-L gathers.
    ps1t = tc.alloc_tile_pool(name="ps1t", bufs=2, space="PSUM")
    for mc in range(CN):
        qht = qht_pool.tile([128, L], F16, tag="qht", name="qht")
        for jg in range(4):
            pt = ps1t.tile([128, 512], F16, tag="pt", name="pt")
            for jj in range(4):
                m = 4 * jg + jj
                nc.tensor.transpose(
                    pt[:, ts(jj, 128)], htd_q[m][:, ts(mc, 128)], ident
                )
            nc.vector.tensor_scalar_add(
                qht[:, ts(jg, 512)], pt[:, :], bqcol[:, mc : mc + 1]
            )
        nc.gpsimd.dma_start(q2[ts(mc, 128), 0:L], qht[:, :])
        nc.gpsimd.dma_start(q2[ts(mc, 128), L : 2 * L], qht[:, :])

    ps1t.release()
    qht_pool.release()
    ps1.release()
    wqk_pool.release()
    qt_pool.release()

    # ---- S3+S4 fused: forward DFT with inline freq product ----
    # Qhat[fs, c] = sum_t Cf[t, fs] * qh_td[t, c]; pairs (j, 9+j) are produced
    # back-to-back so Z = Qhat * conj(Khat) is computed inline and the big
    # Qhat/Khat buffers never materialize.
    s_pool0 = tc.alloc_tile_pool(name="small0", bufs=1)
    iobs = []
    for mc in range(CN):
        iob = s_pool0.tile([128, 8], U32, tag=f"io{mc}", name=f"io{mc}")
        nc.gpsimd.iota(
            iob[:, :], pattern=[[0, 8]], base=mc * 128 * 2 * L,
            channel_multiplier=2 * L,
        )
        iobs.append(iob)
    # resident inverse basis: 4 fp16 n-chunks (8 MB), loaded on the Act HWDGE
    # queue while the forward DFT runs.
    mi_pool = tc.alloc_tile_pool(name="mi", bufs=1)
    mi_t = [mi_pool.tile([128, FM * 512], F16, tag=f"mi{n}", name=f"mi{n}")
            for n in range(4)]
    for n in range(4):
        nc.scalar.dma_start(mi_t[n][:, :], dr["Mi"][ts(n, 128), :])

    z_pool = tc.alloc_tile_pool(name="zfreq", bufs=1)
    f_pool = tc.alloc_tile_pool(name="fpair", bufs=4)
    ps3 = tc.alloc_tile_pool(name="ps3", bufs=2, space="PSUM")

    Z = [z_pool.tile([128, D], F16, tag=f"z{j}", name=f"z{j}") for j in range(FM)]

    def dft_chunk(m, scale):
        psq = ps3.tile([128, D], F32, tag="p3q", name="p3q")
        psk = ps3.tile([128, D], F32, tag="p3k", name="p3k")
        cf_t = cf_pool.tile([128, TM * 128], F16, tag="cf", name="cf")
        nc.sync.dma_start(cf_t[:, :], dr["Cf"][ts(m, 128), :])
        for kc in range(TM):
            nc.tensor.matmul(
                psq[:, :], cf_t[:, ts(kc, 128)], htd_q[kc][:, :],
                start=(kc == 0), stop=(kc == TM - 1),
            )
            nc.tensor.matmul(
                psk[:, :], cf_t[:, ts(kc, 128)], htd_k[kc][:, :],
                start=(kc == 0), stop=(kc == TM - 1),
            )
        qf = f_pool.tile([128, D], F32R, tag="qf", name="qf")
        kf = f_pool.tile([128, D], F32R, tag="kf", name="kf")
        nc.scalar.activation(qf[:, :], psq[:, :], AF.Copy)
        # fold the 2/L irfft scale into the k spectrum
        nc.scalar.activation(kf[:, :], psk[:, :], AF.Copy, scale=scale)
        return qf, kf

    for j in range(8):
        re, im = j, 8 + j
        qf_a, kf_a = dft_chunk(re, 2.0 / L)
        if j == 0:
            nc.vector.tensor_add(qf_a[0:1, :], qf_a[0:1, :], brow["bqL"][:, :])
            nc.vector.tensor_add(kf_a[0:1, :], kf_a[0:1, :], brow["bkL2"][:, :])
        qf_b, kf_b = dft_chunk(im, 2.0 / L)
        # Zre_j = Qre Kre + Qnim Knim ; Znim_j = Qnim Kre - Qre Knim
        t0 = f_pool.tile([128, D], F32R, tag="zt", name="zt")
        t1 = f_pool.tile([128, D], F32R, tag="zt", name="zt")
        nc.vector.tensor_mul(t0[:, :], qf_a[:, :], kf_a[:, :])
        nc.gpsimd.tensor_mul(t1[:, :], qf_b[:, :], kf_b[:, :])
        nc.vector.tensor_add(Z[re][:, :], t0[:, :], t1[:, :])
        t2 = f_pool.tile([128, D], F32R, tag="zt", name="zt")
        t3 = f_pool.tile([128, D], F32R, tag="zt", name="zt")
        nc.gpsimd.tensor_mul(t2[:, :], qf_b[:, :], kf_a[:, :])
        nc.vector.tensor_mul(t3[:, :], qf_a[:, :], kf_b[:, :])
        nc.vector.tensor_sub(Z[im][:, :], t2[:, :], t3[:, :])
        if j == 0:
            # row 0 of chunk 0 is the DC bin (no im partner): Z = Q0 * K0.
            # row 0 of chunk 8 is the Nyquist bin (real): Z = QN * KN.
            # Both are 1/L-scaled bins; kf carries 2/L, so halve.
            nc.vector.tensor_mul(t0[0:1, :], qf_a[0:1, :], kf_a[0:1, :])
            nc.vector.tensor_scalar_mul(Z[re][0:1, :], t0[0:1, :], 0.5)
            nc.vector.tensor_mul(t1[0:1, :], qf_b[0:1, :], kf_b[0:1, :])
            nc.vector.tensor_scalar_mul(Z[im][0:1, :], t1[0:1, :], 0.5)

    ps3.release()
    f_pool.release()
    htd_pool.release()
    cf_pool.release()

    # ---- S5/S6/S7 interleaved per channel chunk ----
    # inv-DFT(mc) on the PE; then its top-k + gather launches (DVE + SWDGE)
    # overlap inv-DFT(mc+1); wsum(mc) fills the PSUM-copy window of
    # inv-DFT(mc+2). Weights are folded into diag(w) fp16 stationaries.
    r_pool = tc.alloc_tile_pool(name="rcorr", bufs=1, side="right")
    psa = tc.alloc_tile_pool(name="psa", bufs=4, space="PSUM")
    ps5 = tc.alloc_tile_pool(name="ps5", bufs=4, space="PSUM")
    s_pool = tc.alloc_tile_pool(name="small", bufs=1)
    acc_pool = tc.alloc_tile_pool(name="acc", bufs=1, side="right")
    g_pool = tc.alloc_tile_pool(name="g", bufs=6)
    dg_pool = tc.alloc_tile_pool(name="dg", bufs=12)

    R = [r_pool.tile([128, L], F32, tag=f"r{m}", name=f"r{m}") for m in range(CN)]
    cand = [s_pool0.tile([128, 32], F32, tag=f"c{m}", name=f"c{m}") for m in range(CN)]
    acc = [acc_pool.tile([128, L], F16, tag=f"a{mc}", name=f"a{mc}") for mc in range(CN)]

    # accumulate in Z-production order (re/im pairs) so the last fwd
    # products are needed last
    KORD = [j for p in range(8) for j in (p, 8 + p)]

    def inv_dft(mc):
        pss = [ps5.tile([128, 512], F32, tag="p5", name="p5") for _ in range(4)]
        for n in range(4):
            for i, kc in enumerate(KORD):
                nc.tensor.matmul(
                    pss[n][:, :], Z[kc][:, ts(mc, 128)], mi_t[n][:, ts(kc, 512)],
                    start=(i == 0), stop=(i == FM - 1),
                )
            nc.scalar.activation(R[mc][:, ts(n, 512)], pss[n][:, :], AF.Copy)
            nc.vector.max(out=cand[mc][:, ts(n, 8)], in_=R[mc][:, ts(n, 512)])

    def topk_gather(mc):
        vals = s_pool.tile([128, 8], F32, tag=f"v{mc}", name=f"v{mc}")
        nc.vector.max(out=vals[:, :], in_=cand[mc][:, :])
        idx = s_pool.tile([128, 8], U32, tag=f"i{mc}", name=f"i{mc}")
        nc.vector.max_index(out=idx[:, :], in_max=vals[:, :], in_values=R[mc][:, :])
        off = s_pool.tile([128, 8], U32, tag=f"o{mc}", name=f"o{mc}")
        nc.vector.tensor_add(off[:, :], idx[:, :], iobs[mc][:, :])
        gs = []
        for k in range(TOPK):
            g = g_pool.tile([128, L], F16, tag="g", name="g")
            gi = nc.gpsimd.indirect_dma_start(
                out=g[:, :],
                out_offset=None,
                in_=q2[:, :],
                in_offset=IndirectOffsetOnAxis(ap=off[:, k : k + 1], axis=1),
            )
            if k % 4:
                gi.ins.queue = f"qPoolDynamic{k % 4}"
            gs.append(g)
        negm = s_pool.tile([128, 1], F32, tag=f"nm{mc}", name=f"nm{mc}")
        nc.vector.tensor_scalar_mul(negm[:, :], vals[:, 0:1], -1.0)
        e = s_pool.tile([128, 8], F32, tag=f"e{mc}", name=f"e{mc}")
        nc.scalar.activation(e[:, :], vals[:, :], AF.Exp, bias=negm[:, :])
        ssum = s_pool.tile([128, 1], F32, tag=f"s{mc}", name=f"s{mc}")
        nc.vector.reduce_sum(out=ssum[:, :], in_=e[:, :], axis=AX.X)
        rs = s_pool.tile([128, 1], F32, tag=f"rs{mc}", name=f"rs{mc}")
        nc.vector.reciprocal(rs[:, :], ssum[:, :])
        wt = s_pool.tile([128, 8], F32, tag=f"w{mc}", name=f"w{mc}")
        nc.vector.tensor_scalar_mul(wt[:, :], e[:, :], rs[:, :])
        ds = []
        for k in range(TOPK):
            dg = dg_pool.tile([128, 128], F16, tag="dg", name="dg")
            nc.vector.tensor_scalar_mul(dg[:, :], ident[:, :], wt[:, k : k + 1])
            ds.append(dg)
        return gs, ds

    def wsum(mc, gs, ds):
        pacc = [psa.tile([128, 512], F32, tag="pa", name="pa") for _ in range(4)]
        for k in range(TOPK):
            for nsl in range(4):
                nc.tensor.matmul(
                    pacc[nsl][:, :], ds[k][:, :], gs[k][:, ts(nsl, 512)],
                    start=(k == 0), stop=(k == TOPK - 1),
                )
        for nsl in range(4):
            nc.scalar.activation(acc[mc][:, ts(nsl, 512)], pacc[nsl][:, :], AF.Copy)

    gd = {}
    inv_dft(0)
    gd[0] = topk_gather(0)
    inv_dft(1)
    gd[1] = topk_gather(1)
    wsum(0, *gd[0])
    inv_dft(2)
    gd[2] = topk_gather(2)
    wsum(1, *gd[1])
    inv_dft(3)
    gd[3] = topk_gather(3)
    wsum(2, *gd[2])

    ps5.release()
    po_pool = tc.alloc_tile_pool(name="po", bufs=1, space="PSUM")
    ot_pool = tc.alloc_tile_pool(name="ot", bufs=4, side="right")

    wsum(3, *gd[3])

    # ---- S8: output projection, TRANSPOSED: outT[c, t] = sum_cin Wo[cin, c]
    # * acc[cin, t] + bo[c]. Channel-major output puts the bias on the
    # partition axis (fused into the PSUM copy); the host un-transposes.
    for cb in range(4):
        pss = [po_pool.tile([128, 512], F32, tag=f"po{tb}", name=f"po{tb}")
               for tb in range(4)]
        for kc in range(CN):
            for tb in range(4):
                nc.tensor.matmul(
                    pss[tb][:, :], wo[kc][:, ts(cb, 128)], acc[kc][:, ts(tb, 512)],
                    start=(kc == 0), stop=(kc == CN - 1),
                )
        for tb in range(4):
            ot = ot_pool.tile([128, 512], F16, tag="ot", name="ot")
            nc.scalar.activation(
                ot[:, :], pss[tb][:, :], AF.Identity, bias=bocol[:, cb : cb + 1]
            )
            eng = nc.sync if tb % 2 == 0 else nc.scalar
            eng.dma_start(out_ap[ts(cb, 128), ts(tb, 512)], ot[:, :])

    ot_pool.release()
    po_pool.release()
    psa.release()
    dg_pool.release()
    g_pool.release()
    s_pool.release()
    z_pool.release()
    mi_pool.release()
    s_pool0.release()
    acc_pool.release()
    r_pool.release()
    w_pool.release()


def build_module():
    nc = bacc.Bacc(
        "TRN2",
        target_bir_lowering=False,
        debug=False,
        enable_asserts=False,
        num_devices=N_CORES,
        num_swdge_queues=4,
    )
    dr = {}

    def din(name, shape, dt=F32R):
        dr[name] = nc.dram_tensor(name, shape, dt, kind="ExternalInput").ap()

    din("qT", [D, L], F16)
    din("kT", [D, L], F16)
    din("Wq", [128, KC * D], F16)   # tiled: [p, kc*D+j] = W[kc*128+p, j]
    din("Wk", [128, KC * D], F16)
    din("Wo", [128, KC * D], F16)
    din("bqL", [1, D])
    din("bkL2", [1, D])
    din("bqc", [D, 1], F32)
    din("boc", [D, 1], F32)
    din("ident", [128, 128], F16)
    din("Cf", [FM * 128, TM * 128], F16)   # [m*128+p, kc*128+j] = Cf[kc*128+p, m*128+j]
    din("Mi", [4 * 128, FM * 512], F16)    # [n*128+p, kc*512+j] = Mi[kc*128+p, n*512+j]
    out_ap = nc.dram_tensor("out", [D, L], F16, kind="ExternalOutput").ap()
    q2 = nc.dram_tensor("q2", [D, 2 * L], F16, kind="Internal").ap()

    with tile.TileContext(nc, trace_sim=False) as tc:
        _kernel_body(tc, dr, out_ap, q2)
    nc.compile()
    return nc


_NC_CACHE = {}


def _tile_w(W):
    return np.ascontiguousarray(
        np.asarray(W, np.float32).reshape(KC, 128, D).transpose(1, 0, 2).reshape(128, KC * D)
    )


def make_in_maps(q, k, Wq, bq, Wk, bk, Wo, bo):
    Cf, Mi = _build_dft_mats()
    # pre-tile so each stage does one big contiguous DMA per chunk column
    Cf = np.ascontiguousarray(
        Cf.reshape(TM, 128, FM, 128).transpose(2, 1, 0, 3).reshape(FM * 128, TM * 128)
    )
    Mi = np.ascontiguousarray(
        Mi.reshape(FM, 128, 4, 512).transpose(2, 1, 0, 3).reshape(4 * 128, FM * 512)
    ).astype(np.float16)
    f32 = np.float32
    shared = {
        "Wq": _tile_w(Wq).astype(np.float16),
        "Wk": _tile_w(Wk).astype(np.float16),
        "Wo": _tile_w(Wo).astype(np.float16),
        "bqL": np.ascontiguousarray(np.asarray(bq, f32) * L, f32).reshape(1, D),
        "bkL2": np.ascontiguousarray(np.asarray(bk, f32) * 2.0, f32).reshape(1, D),
        "bqc": np.ascontiguousarray(bq, f32).reshape(D, 1),
        "boc": np.ascontiguousarray(bo, f32).reshape(D, 1),
        "ident": np.eye(128, dtype=np.float16),
        "Cf": Cf.astype(np.float16),
        "Mi": Mi,
    }
    in_maps = []
    for b in range(B):
        m = dict(shared)
        m["qT"] = np.ascontiguousarray(np.asarray(q[b], f32).T).astype(np.float16)
        m["kT"] = np.ascontiguousarray(np.asarray(k[b], f32).T).astype(np.float16)
        in_maps.append(m)
    return in_maps


def kernel(q, k, v, Wq, bq, Wk, bk, Wv, bv, Wo, bo, _want_results=False,
           _trace=False, **_ignored):
    if "nc" not in _NC_CACHE:
        _NC_CACHE["nc"] = build_module()
    nc = _NC_CACHE["nc"]
    in_maps = make_in_maps(q, k, Wq, bq, Wk, bk, Wo, bo)
    res = run_bass_kernel_spmd(
        nc, in_maps, core_ids=list(range(N_CORES)), trace=_trace
    )
    out = np.stack([np.asarray(res.results[b]["out"], np.float32).T for b in range(B)])
    out = np.ascontiguousarray(out)
    if _want_results:
        return out, res
    return out


if __name__ == "__main__":
    # smoke test with random data
    rng = np.random.default_rng(0)
    q = rng.standard_normal((B, L, D), np.float32)
    k = rng.standard_normal((B, L, D), np.float32)
    s = 1.0 / np.sqrt(D)
    Wq = rng.standard_normal((D, D), np.float32) * s
    Wk = rng.standard_normal((D, D), np.float32) * s
    Wo = rng.standard_normal((D, D), np.float32) * s
    z = np.zeros(D, np.float32)
    out = kernel(q, k, None, Wq, z, Wk, z, None, None, Wo, z)
    print("out", out.shape, out.dtype, float(np.abs(out).sum()))



# revision 32
# speedup vs baseline: 95172.1303x; 1.0048x over previous
"""AutoCorrelation block (FFT cross-correlation attention) on 8 Trainium2 cores.

Math (per batch b, faithfully reproducing the reference):
  qh = q @ Wq + bq, kh = k @ Wk + bk         (v projection is dead code)
  per channel c=(h,dh) (512 per batch):
    r = irfft(rfft(qh_c) * conj(rfft(kh_c)))   # circular cross-correlation
    top-8 lags d_k of r, softmax of the 8 values -> w_k
    agg_c[t] = sum_k w_k * qh_c[(t + d_k) % L]
  out = agg^T @ Wo + bo

Implementation: DFT-as-matmul with a stacked real cos/sin basis (the DFT matrix
is shared by all channels, so the whole FFT pipeline is dense PE work), DVE
max/max_index for top-8, and per-partition indirect-DMA gathers from a
time-doubled copy of qh for the mod-L rolls.

Sharding: data-parallel over batch. B == 8 == n_cores, one batch per core,
weights + DFT matrices replicated. No collectives.
"""

import numpy as np

import concourse.bass as bass
import concourse.bacc as bacc
import concourse.mybir as mybir
import concourse.tile as tile
from concourse.bass import IndirectOffsetOnAxis, ts
from concourse.bass_utils import run_bass_kernel_spmd

B, L, D = 8, 2048, 512
TOPK = 8
NF = 1025          # rfft bins for L=2048
FS = 2048          # stacked freq rows: 16 chunks of 128
IM0 = 1024         # sin(f) block at 1024+f (f=1..1023); slot 1024 = Nyquist cos
N_CORES = 8
KC = 4             # d_in chunks of 128
TM = 16            # time chunks of 128
CN = 4             # channel chunks of 128
FM = 16            # stacked-freq chunks of 128

F32 = mybir.dt.float32
F32R = mybir.dt.float32r
U32 = mybir.dt.uint32
BF16 = mybir.dt.bfloat16
F16 = mybir.dt.float16
AF = mybir.ActivationFunctionType
AX = mybir.AxisListType


def _build_dft_mats():
    # 16-chunk stacked real basis: cols 0..1023 = cos(2pi f t/L); col 1024 =
    # (-1)^t (Nyquist, reusing the identically-zero sin(0) slot); cols 1024+f =
    # sin(2pi f t/L) for f=1..1023. The frequency product treats chunk pairs
    # (j, 8+j) as (re, im); rows 0 and 1024 of Z get small post-fixes.
    t = np.arange(L)
    f = np.arange(1024)
    ang = (2.0 * np.pi / L) * ((t[:, None] * f[None, :]) % L)
    Cf = np.zeros((L, FS), np.float32)
    Cf[:, :1024] = np.cos(ang)
    Cf[:, 1024] = np.where(t % 2 == 0, 1.0, -1.0)
    Cf[:, 1025:] = np.sin(ang[:, 1:])
    # Mi is the UNSCALED inverse basis (entries in [-1, 1], exact in fp16);
    # the 2/L irfft scale is folded into the kf copy on-device (DC/Nyquist
    # rows get an extra 0.5 in the Z fix-up).
    ang2 = (2.0 * np.pi / L) * ((f[:, None] * t[None, :]) % L)
    Mi = np.zeros((FS, L), np.float32)
    Mi[0, :] = 1.0
    Mi[1:1024, :] = np.cos(ang2[1:])
    Mi[1024, :] = np.where(t % 2 == 0, 1.0, -1.0)
    Mi[1025:, :] = np.sin(ang2[1:])
    return Cf, Mi


def _kernel_body(tc, dr, out_ap, q2):
    nc = tc.nc

    w_pool = tc.alloc_tile_pool(name="weights", bufs=1)
    cf_pool = tc.alloc_tile_pool(name="cf", bufs=4, side="right")
    htd_pool = tc.alloc_tile_pool(name="htd", bufs=1, side="right")

    # ---- S1 inputs first so the PE can start ASAP ----
    qt_pool = tc.alloc_tile_pool(name="qt", bufs=1)
    qt = [qt_pool.tile([128, L], F16, tag=f"qt{i}", name=f"qt{i}") for i in range(KC)]
    kt = [qt_pool.tile([128, L], F16, tag=f"kt{i}", name=f"kt{i}") for i in range(KC)]

    # ---- constants (DMA order matters: the sync queue is in-order, so load
    # exactly what the first matmul group needs first) ----
    wqk_pool = tc.alloc_tile_pool(name="wqk", bufs=1)
    wq_t = wqk_pool.tile([128, KC * D], F16, tag="wqt", name="wqt")
    wk_t = wqk_pool.tile([128, KC * D], F16, tag="wkt", name="wkt")
    wo_t = w_pool.tile([128, KC * D], F16, tag="wot", name="wot")
    # tiny constants first (37 KB): bias rows and ident must not sit behind
    # megabyte loads — the grp-0 bias matmul needs them at ~14us.
    ident = w_pool.tile([128, 128], F16, tag="ident", name="ident")
    nc.sync.dma_start(ident[:, :], dr["ident"][:, :])
    brow = {}
    for nm in ("bqL", "bkL2"):
        brow[nm] = w_pool.tile([1, D], F32R, tag=f"{nm}r", name=f"{nm}r")
        nc.sync.dma_start(brow[nm][:, :], dr[nm][:, :])
    bqcol = w_pool.tile([128, CN], F32, tag="bqc", name="bqc")
    for c in range(CN):
        nc.sync.dma_start(bqcol[:, c : c + 1], dr["bqc"][ts(c, 128), :])
    bocol = w_pool.tile([128, CN], F32, tag="boc", name="boc")
    for c in range(CN):
        nc.sync.dma_start(bocol[:, c : c + 1], dr["boc"][ts(c, 128), :])
    # quarter-tile interleaved loads: the first matmul group needs ~0.8 MB
    for i in range(KC):
        nc.sync.dma_start(qt[i][:, 0:512], dr["qT"][ts(i, 128), 0:512])
        nc.sync.dma_start(wq_t[:, ts(i, D)], dr["Wq"][:, ts(i, D)])
    for q4 in range(1, 4):
        for i in range(KC):
            nc.sync.dma_start(qt[i][:, ts(q4, 512)], dr["qT"][ts(i, 128), ts(q4, 512)])
    # k-side after q-side on the same queue: full bandwidth for the critical
    # path, and kt still lands well before the kh projection (~31us)
    for i in range(KC):
        nc.sync.dma_start(wk_t[:, ts(i, D)], dr["Wk"][:, ts(i, D)])
        nc.sync.dma_start(kt[i][:, 0:512], dr["kT"][ts(i, 128), 0:512])
    for q4 in range(1, 4):
        for i in range(KC):
            nc.sync.dma_start(kt[i][:, ts(q4, 512)], dr["kT"][ts(i, 128), ts(q4, 512)])
    nc.scalar.dma_start(wo_t[:, :], dr["Wo"][:, :])
    wq = [wq_t[:, ts(i, D)] for i in range(KC)]
    wk = [wk_t[:, ts(i, D)] for i in range(KC)]
    wo = [wo_t[:, ts(i, D)] for i in range(KC)]

    htd_q = [htd_pool.tile([128, D], F16, tag=f"hq{m}", name=f"hq{m}") for m in range(TM)]
    htd_k = [htd_pool.tile([128, D], F16, tag=f"hk{m}", name=f"hk{m}") for m in range(TM)]

    # ---- S1/S2: projections (all-fp16 operands, fp32 PSUM accumulate) ----
    ps1 = tc.alloc_tile_pool(name="ps1", bufs=6, space="PSUM")
    qht_pool = tc.alloc_tile_pool(name="qht", bufs=2)

    # qh_td[t, c] = sum_di qT[di, t] * Wq[di, c]; bias is applied in the
    # channel-major transpose copies (per-partition there) and via the DC-bin
    # fix in the forward DFT.
    for grp in range(4):
        pss1 = [ps1.tile([128, D], F32, tag="p1", name="p1") for _ in range(4)]
        for kc in range(KC):
            for m4 in range(4):
                nc.tensor.matmul(
                    pss1[m4][:, :], qt[kc][:, ts(grp * 4 + m4, 128)], wq[kc],
                    start=(kc == 0), stop=(kc == KC - 1),
                )
        for m4 in range(4):
            nc.scalar.activation(
                htd_q[grp * 4 + m4][:, :], pss1[m4][:, :], AF.Copy
            )
    # kh_td
    for m in range(TM):
        ps = ps1.tile([128, D], F32, tag="p1", name="p1")
        for kc in range(KC):
            nc.tensor.matmul(
                ps[:, :], kt[kc][:, ts(m, 128)], wk[kc],
                start=(kc == 0), stop=(kc == KC - 1),
            )
        nc.scalar.activation(htd_k[m][:, :], ps[:, :], AF.Copy)
    # qh_t[c, t] channel-major via PE transposes of the fp16 htd tiles
    # (1 cycle/row, 6x cheaper than re-projecting), DVE drains PSUM, then
    # doubled into q2 for the mod-L gathers.
    ps1t = tc.alloc_tile_pool(name="ps1t", bufs=2, space="PSUM")
    for mc in range(CN):
        qht = qht_pool.tile([128, L], F16, tag="qht", name="qht")
        for jg in range(4):
            pt = ps1t.tile([128, 512], F16, tag="pt", name="pt")
            for jj in range(4):
                m = 4 * jg + jj
                nc.tensor.transpose(
                    pt[:, ts(jj, 128)], htd_q[m][:, ts(mc, 128)], ident
                )
            nc.vector.tensor_scalar_add(
                qht[:, ts(jg, 512)], pt[:, :], bqcol[:, mc : mc + 1]
            )
        nc.gpsimd.dma_start(q2[ts(mc, 128), 0:L], qht[:, :])
        nc.gpsimd.dma_start(q2[ts(mc, 128), L : 2 * L], qht[:, :])

    ps1t.release()
    qht_pool.release()
    ps1.release()
    wqk_pool.release()
    qt_pool.release()

    # ---- S3+S4 fused: forward DFT with inline freq product ----
    # Qhat[fs, c] = sum_t Cf[t, fs] * qh_td[t, c]; pairs (j, 9+j) are produced
    # back-to-back so Z = Qhat * conj(Khat) is computed inline and the big
    # Qhat/Khat buffers never materialize.
    s_pool0 = tc.alloc_tile_pool(name="small0", bufs=1)
    iobs = []
    for mc in range(CN):
        iob = s_pool0.tile([128, 8], U32, tag=f"io{mc}", name=f"io{mc}")
        nc.gpsimd.iota(
            iob[:, :], pattern=[[0, 8]], base=mc * 128 * 2 * L,
            channel_multiplier=2 * L,
        )
        iobs.append(iob)
    # resident inverse basis: 4 fp16 n-chunks (8 MB), loaded on the Act HWDGE
    # queue while the forward DFT runs.
    mi_pool = tc.alloc_tile_pool(name="mi", bufs=1)
    mi_t = [mi_pool.tile([128, FM * 512], F16, tag=f"mi{n}", name=f"mi{n}")
            for n in range(4)]
    for n in range(4):
        nc.scalar.dma_start(mi_t[n][:, :], dr["Mi"][ts(n, 128), :])

    z_pool = tc.alloc_tile_pool(name="zfreq", bufs=1)
    f_pool = tc.alloc_tile_pool(name="fpair", bufs=4)
    ps3 = tc.alloc_tile_pool(name="ps3", bufs=2, space="PSUM")

    Z = [z_pool.tile([128, D], F16, tag=f"z{j}", name=f"z{j}") for j in range(FM)]

    def dft_chunk(m, scale):
        psq = ps3.tile([128, D], F32, tag="p3q", name="p3q")
        psk = ps3.tile([128, D], F32, tag="p3k", name="p3k")
        cf_t = cf_pool.tile([128, TM * 128], F16, tag="cf", name="cf")
        nc.sync.dma_start(cf_t[:, :], dr["Cf"][ts(m, 128), :])
        for kc in range(TM):
            nc.tensor.matmul(
                psq[:, :], cf_t[:, ts(kc, 128)], htd_q[kc][:, :],
                start=(kc == 0), stop=(kc == TM - 1),
            )
            nc.tensor.matmul(
                psk[:, :], cf_t[:, ts(kc, 128)], htd_k[kc][:, :],
                start=(kc == 0), stop=(kc == TM - 1),
            )
        qf = f_pool.tile([128, D], F32R, tag="qf", name="qf")
        kf = f_pool.tile([128, D], F32R, tag="kf", name="kf")
        nc.scalar.activation(qf[:, :], psq[:, :], AF.Copy)
        # fold the 2/L irfft scale into the k spectrum
        nc.scalar.activation(kf[:, :], psk[:, :], AF.Copy, scale=scale)
        return qf, kf

    for j in range(8):
        re, im = j, 8 + j
        qf_a, kf_a = dft_chunk(re, 2.0 / L)
        if j == 0:
            nc.vector.tensor_add(qf_a[0:1, :], qf_a[0:1, :], brow["bqL"][:, :])
            nc.vector.tensor_add(kf_a[0:1, :], kf_a[0:1, :], brow["bkL2"][:, :])
        qf_b, kf_b = dft_chunk(im, 2.0 / L)
        # Zre_j = Qre Kre + Qnim Knim ; Znim_j = Qnim Kre - Qre Knim
        t0 = f_pool.tile([128, D], F32R, tag="zt", name="zt")
        t1 = f_pool.tile([128, D], F32R, tag="zt", name="zt")
        nc.vector.tensor_mul(t0[:, :], qf_a[:, :], kf_a[:, :])
        nc.gpsimd.tensor_mul(t1[:, :], qf_b[:, :], kf_b[:, :])
        nc.vector.tensor_add(Z[re][:, :], t0[:, :], t1[:, :])
        t2 = f_pool.tile([128, D], F32R, tag="zt", name="zt")
        t3 = f_pool.tile([128, D], F32R, tag="zt", name="zt")
        nc.gpsimd.tensor_mul(t2[:, :], qf_b[:, :], kf_a[:, :])
        nc.vector.tensor_mul(t3[:, :], qf_a[:, :], kf_b[:, :])
        nc.vector.tensor_sub(Z[im][:, :], t2[:, :], t3[:, :])
        if j == 0:
            # row 0 of chunk 0 is the DC bin (no im partner): Z = Q0 * K0.
            # row 0 of chunk 8 is the Nyquist bin (real): Z = QN * KN.
            # Both are 1/L-scaled bins; kf carries 2/L, so halve.
            nc.vector.tensor_mul(t0[0:1, :], qf_a[0:1, :], kf_a[0:1, :])
            nc.vector.tensor_scalar_mul(Z[re][0:1, :], t0[0:1, :], 0.5)
            nc.vector.tensor_mul(t1[0:1, :], qf_b[0:1, :], kf_b[0:1, :])
            nc.vector.tensor_scalar_mul(Z[im][0:1, :], t1[0:1, :], 0.5)

    ps3.release()
    f_pool.release()
    htd_pool.release()
    cf_pool.release()

    # ---- S5/S6/S7 interleaved per channel chunk ----
    # inv-DFT(mc) on the PE; then its top-k + gather launches (DVE + SWDGE)
    # overlap inv-DFT(mc+1); wsum(mc) fills the PSUM-copy window of
    # inv-DFT(mc+2). Weights are folded into diag(w) fp16 stationaries.
    r_pool = tc.alloc_tile_pool(name="rcorr", bufs=1, side="right")
    psa = tc.alloc_tile_pool(name="psa", bufs=4, space="PSUM")
    ps5 = tc.alloc_tile_pool(name="ps5", bufs=4, space="PSUM")
    s_pool = tc.alloc_tile_pool(name="small", bufs=1)
    acc_pool = tc.alloc_tile_pool(name="acc", bufs=1, side="right")
    g_pool = tc.alloc_tile_pool(name="g", bufs=6)
    dg_pool = tc.alloc_tile_pool(name="dg", bufs=12)

    R = [r_pool.tile([128, L], F32, tag=f"r{m}", name=f"r{m}") for m in range(CN)]
    cand = [s_pool0.tile([128, 32], F32, tag=f"c{m}", name=f"c{m}") for m in range(CN)]
    acc = [acc_pool.tile([128, L], F16, tag=f"a{mc}", name=f"a{mc}") for mc in range(CN)]

    # accumulate in Z-production order (re/im pairs) so the last fwd
    # products are needed last
    KORD = [j for p in range(8) for j in (p, 8 + p)]

    def inv_dft(mc):
        pss = [ps5.tile([128, 512], F32, tag="p5", name="p5") for _ in range(4)]
        for n in range(4):
            for i, kc in enumerate(KORD):
                nc.tensor.matmul(
                    pss[n][:, :], Z[kc][:, ts(mc, 128)], mi_t[n][:, ts(kc, 512)],
                    start=(i == 0), stop=(i == FM - 1),
                )
            nc.scalar.activation(R[mc][:, ts(n, 512)], pss[n][:, :], AF.Copy)
            nc.vector.max(out=cand[mc][:, ts(n, 8)], in_=R[mc][:, ts(n, 512)])

    def topk_gather(mc):
        vals = s_pool.tile([128, 8], F32, tag=f"v{mc}", name=f"v{mc}")
        nc.vector.max(out=vals[:, :], in_=cand[mc][:, :])
        idx = s_pool.tile([128, 8], U32, tag=f"i{mc}", name=f"i{mc}")
        nc.vector.max_index(out=idx[:, :], in_max=vals[:, :], in_values=R[mc][:, :])
        off = s_pool.tile([128, 8], U32, tag=f"o{mc}", name=f"o{mc}")
        nc.vector.tensor_add(off[:, :], idx[:, :], iobs[mc][:, :])
        gs = []
        for k in range(TOPK):
            g = g_pool.tile([128, L], F16, tag="g", name="g")
            gi = nc.gpsimd.indirect_dma_start(
                out=g[:, :],
                out_offset=None,
                in_=q2[:, :],
                in_offset=IndirectOffsetOnAxis(ap=off[:, k : k + 1], axis=1),
            )
            if k % 4:
                gi.ins.queue = f"qPoolDynamic{k % 4}"
            gs.append(g)
        negm = s_pool.tile([128, 1], F32, tag=f"nm{mc}", name=f"nm{mc}")
        nc.vector.tensor_scalar_mul(negm[:, :], vals[:, 0:1], -1.0)
        e = s_pool.tile([128, 8], F32, tag=f"e{mc}", name=f"e{mc}")
        nc.scalar.activation(e[:, :], vals[:, :], AF.Exp, bias=negm[:, :])
        ssum = s_pool.tile([128, 1], F32, tag=f"s{mc}", name=f"s{mc}")
        nc.vector.reduce_sum(out=ssum[:, :], in_=e[:, :], axis=AX.X)
        rs = s_pool.tile([128, 1], F32, tag=f"rs{mc}", name=f"rs{mc}")
        nc.vector.reciprocal(rs[:, :], ssum[:, :])
        wt = s_pool.tile([128, 8], F32, tag=f"w{mc}", name=f"w{mc}")
        nc.vector.tensor_scalar_mul(wt[:, :], e[:, :], rs[:, :])
        ds = []
        for k in range(TOPK):
            dg = dg_pool.tile([128, 128], F16, tag="dg", name="dg")
            nc.vector.tensor_scalar_mul(dg[:, :], ident[:, :], wt[:, k : k + 1])
            ds.append(dg)
        return gs, ds

    def wsum(mc, gs, ds):
        pacc = [psa.tile([128, 512], F32, tag="pa", name="pa") for _ in range(4)]
        for k in range(TOPK):
            for nsl in range(4):
                nc.tensor.matmul(
                    pacc[nsl][:, :], ds[k][:, :], gs[k][:, ts(nsl, 512)],
                    start=(k == 0), stop=(k == TOPK - 1),
                )
        for nsl in range(4):
            nc.scalar.activation(acc[mc][:, ts(nsl, 512)], pacc[nsl][:, :], AF.Copy)

    gd = {}
    inv_dft(0)
    gd[0] = topk_gather(0)
    inv_dft(1)
    gd[1] = topk_gather(1)
    wsum(0, *gd[0])
    inv_dft(2)
    gd[2] = topk_gather(2)
    wsum(1, *gd[1])
    inv_dft(3)
    gd[3] = topk_gather(3)
    wsum(2, *gd[2])

    ps5.release()
    po_pool = tc.alloc_tile_pool(name="po", bufs=1, space="PSUM")
    ot_pool = tc.alloc_tile_pool(name="ot", bufs=4, side="right")

    wsum(3, *gd[3])

    # ---- S8: output projection, TRANSPOSED: outT[c, t] = sum_cin Wo[cin, c]
    # * acc[cin, t] + bo[c]. Channel-major output puts the bias on the
    # partition axis (fused into the PSUM copy); the host un-transposes.
    for cb in range(4):
        pss = [po_pool.tile([128, 512], F32, tag=f"po{tb}", name=f"po{tb}")
               for tb in range(4)]
        for kc in range(CN):
            for tb in range(4):
                nc.tensor.matmul(
                    pss[tb][:, :], wo[kc][:, ts(cb, 128)], acc[kc][:, ts(tb, 512)],
                    start=(kc == 0), stop=(kc == CN - 1),
                )
        for tb in range(4):
            ot = ot_pool.tile([128, 512], F16, tag="ot", name="ot")
            nc.scalar.activation(
                ot[:, :], pss[tb][:, :], AF.Identity, bias=bocol[:, cb : cb + 1]
            )
            eng = nc.sync if tb % 2 == 0 else nc.scalar
            eng.dma_start(out_ap[ts(cb, 128), ts(tb, 512)], ot[:, :])

    ot_pool.release()
    po_pool.release()
    psa.release()
    dg_pool.release()
    g_pool.release()
    s_pool.release()
    z_pool.release()
    mi_pool.release()
    s_pool0.release()
    acc_pool.release()
    r_pool.release()
    w_pool.release()


def build_module():
    nc = bacc.Bacc(
        "TRN2",
        target_bir_lowering=False,
        debug=False,
        enable_asserts=False,
        num_devices=N_CORES,
        num_swdge_queues=4,
    )
    dr = {}

    def din(name, shape, dt=F32R):
        dr[name] = nc.dram_tensor(name, shape, dt, kind="ExternalInput").ap()

    din("qT", [D, L], F16)
    din("kT", [D, L], F16)
    din("Wq", [128, KC * D], F16)   # tiled: [p, kc*D+j] = W[kc*128+p, j]
    din("Wk", [128, KC * D], F16)
    din("Wo", [128, KC * D], F16)
    din("bqL", [1, D])
    din("bkL2", [1, D])
    din("bqc", [D, 1], F32)
    din("boc", [D, 1], F32)
    din("ident", [128, 128], F16)
    din("Cf", [FM * 128, TM * 128], F16)   # [m*128+p, kc*128+j] = Cf[kc*128+p, m*128+j]
    din("Mi", [4 * 128, FM * 512], F16)    # [n*128+p, kc*512+j] = Mi[kc*128+p, n*512+j]
    out_ap = nc.dram_tensor("out", [D, L], F16, kind="ExternalOutput").ap()
    q2 = nc.dram_tensor("q2", [D, 2 * L], F16, kind="Internal").ap()

    with tile.TileContext(nc, trace_sim=False) as tc:
        _kernel_body(tc, dr, out_ap, q2)
    nc.compile()
    return nc


_NC_CACHE = {}


def _tile_w(W):
    return np.ascontiguousarray(
        np.asarray(W, np.float32).reshape(KC, 128, D).transpose(1, 0, 2).reshape(128, KC * D)
    )


def make_in_maps(q, k, Wq, bq, Wk, bk, Wo, bo):
    Cf, Mi = _build_dft_mats()
    # pre-tile so each stage does one big contiguous DMA per chunk column
    Cf = np.ascontiguousarray(
        Cf.reshape(TM, 128, FM, 128).transpose(2, 1, 0, 3).reshape(FM * 128, TM * 128)
    )
    Mi = np.ascontiguousarray(
        Mi.reshape(FM, 128, 4, 512).transpose(2, 1, 0, 3).reshape(4 * 128, FM * 512)
    ).astype(np.float16)
    f32 = np.float32
    shared = {
        "Wq": _tile_w(Wq).astype(np.float16),
        "Wk": _tile_w(Wk).astype(np.float16),
        "Wo": _tile_w(Wo).astype(np.float16),
        "bqL": np.ascontiguousarray(np.asarray(bq, f32) * L, f32).reshape(1, D),
        "bkL2": np.ascontiguousarray(np.asarray(bk, f32) * 2.0, f32).reshape(1, D),
        "bqc": np.ascontiguousarray(bq, f32).reshape(D, 1),
        "boc": np.ascontiguousarray(bo, f32).reshape(D, 1),
        "ident": np.eye(128, dtype=np.float16),
        "Cf": Cf.astype(np.float16),
        "Mi": Mi,
    }
    in_maps = []
    for b in range(B):
        m = dict(shared)
        m["qT"] = np.ascontiguousarray(np.asarray(q[b], f32).T).astype(np.float16)
        m["kT"] = np.ascontiguousarray(np.asarray(k[b], f32).T).astype(np.float16)
        in_maps.append(m)
    return in_maps


def kernel(q, k, v, Wq, bq, Wk, bk, Wv, bv, Wo, bo, _want_results=False,
           _trace=False, **_ignored):
    if "nc" not in _NC_CACHE:
        _NC_CACHE["nc"] = build_module()
    nc = _NC_CACHE["nc"]
    in_maps = make_in_maps(q, k, Wq, bq, Wk, bk, Wo, bo)
    res = run_bass_kernel_spmd(
        nc, in_maps, core_ids=list(range(N_CORES)), trace=_trace
    )
    out = np.stack([np.asarray(res.results[b]["out"], np.float32).T for b in range(B)])
    out = np.ascontiguousarray(out)
    if _want_results:
        return out, res
    return out


if __name__ == "__main__":
    # smoke test with random data
    rng = np.random.default_rng(0)
    q = rng.standard_normal((B, L, D), np.float32)
    k = rng.standard_normal((B, L, D), np.float32)
    s = 1.0 / np.sqrt(D)
    Wq = rng.standard_normal((D, D), np.float32) * s
    Wk = rng.standard_normal((D, D), np.float32) * s
    Wo = rng.standard_normal((D, D), np.float32) * s
    z = np.zeros(D, np.float32)
    out = kernel(q, k, None, Wq, z, Wk, z, None, None, Wo, z)
    print("out", out.shape, out.dtype, float(np.abs(out).sum()))

